# revision 36
# baseline (speedup 1.0000x reference)
"""Multi-head attention Trainium2 kernel, 8-way sharded, mask-compacted keys.

Problem: x[4,2048,1024] -> qkv proj (w_qkv [3072,1024]) -> 16-head attention
with key-padding mask -> tail proj (w_tail [1024,1024]) + b_tail.

Sharding: 8 shards = 4 batches x 2 head-groups (8 heads each). Each core
computes, for its (batch b, head-group hg):
  - q projection of x[b] (all T tokens) for its 8 heads
  - k/v projections of the mask-COMPACTED tokens of x[b] (keys with mask=0
    contribute exp(-inf)=0 to softmax, so they are dropped host-side and
    the key axis padded to KPAD, a multiple of 128; pads get bias -8e9)
  - [T x KPAD] masked attention per head
  - partial tail matmul y_part = attn_cat @ w_tail[:, cat_slice].T
Host unshards: out[b] = y_part[2b] + y_part[2b+1] + b_tail.  No collectives.

Engine strategy (trace-driven):
  - phase 1 (projections) and phase 3 (tail) are PE-dense: bf16 operands
    (1 cyc/row, background weight loads), PSUM->SBUF copies alternate
    between DVE and ACT so neither serializes the PE.
  - phase 2 (attention) is paced by ACT exp ([128,1024] tiles); PE operands
    stay float32r: the serial 4-byte weight load pads PE occupancy to
    ~match ACT, keeping the HAM clock at 8/8 (bf16 here made the PE idle
    23% per kb and the HAM halved the clock for ~180us).
"""

import time as _time

import numpy as np
from contextlib import ExitStack

import concourse.bass as bass
import concourse.mybir as mybir
import concourse.tile as tile
from concourse.bass_utils import run_bass_kernel_spmd

# ---------------------------------------------------------------------------
# walrus in this env accepts at most 2 sync waits per instruction; Tile's
# scheduler emits up to 10. Post-pass: peel excess waits onto same-engine
# NoOps inserted immediately before the offending instruction (same engine
# stream position => identical synchronization semantics).
MAX_WAITS = 1


def split_excess_waits(nc):
    for fn in nc.m.functions:
        for bb in fn.blocks:
            insts = list(bb.instructions)
            out = []
            changed = False
            for inst in insts:
                si = inst.sync_info
                waits = list(si.on_wait) if si is not None else []
                if len(waits) > MAX_WAITS:
                    extra = waits[:-MAX_WAITS]
                    for ci in range(0, len(extra), MAX_WAITS):
                        chunk = extra[ci:ci + MAX_WAITS]
                        nop = mybir.InstNoOp(
                            name=f"{inst.name}-ws{ci}", ins=[], outs=[])
                        nop.engine = inst.engine
                        nop.sync_info = mybir.SyncInfo(
                            on_wait=chunk, on_update=[])
                        out.append(nop)
                    inst.sync_info = mybir.SyncInfo(
                        on_wait=waits[-MAX_WAITS:],
                        on_update=list(si.on_update))
                    changed = True
                out.append(inst)
            if changed:
                bb.instructions = out
# ---------------------------------------------------------------------------

D_MODEL = 1024
N_HEAD = 16
D_HEAD = 64
BN, T = 4, 2048
HPC = 8                      # heads per core
NPAIR = HPC // 2             # head pairs (q/k tiles hold 2 heads)
CAT = HPC * D_HEAD           # 512 per-core tail contraction
NTB = T // 128               # 16 query-token blocks
QH = T // 2                  # 1024, q processed in two halves
KC = D_MODEL // 128          # 8 contraction chunks
F32 = mybir.dt.float32
F32R = mybir.dt.float32r
BF16 = mybir.dt.bfloat16
I32 = mybir.dt.int32


def build_nc(kpad, split_waits=True):
    assert kpad % 128 == 0 and 128 <= kpad <= T
    NKB = kpad // 128        # key blocks
    LAG = min(4, NKB - 1) if NKB > 1 else 0
    # k-projection chunks: (start, width), width 512 or the tail remainder
    KCH = [(c * 512, min(512, kpad - c * 512))
           for c in range((kpad + 511) // 512)]

    nc = bass.Bass()
    xT = nc.declare_dram_parameter("xT", [D_MODEL, T], BF16, isOutput=False)
    xkT = nc.declare_dram_parameter("xkT", [D_MODEL, kpad], BF16, isOutput=False)
    wqT = nc.declare_dram_parameter("wqT", [D_MODEL, CAT], BF16, isOutput=False)
    wkT = nc.declare_dram_parameter("wkT", [D_MODEL, CAT], BF16, isOutput=False)
    wvT = nc.declare_dram_parameter("wvT", [D_MODEL, CAT], BF16, isOutput=False)
    wtailT = nc.declare_dram_parameter("wtailT", [CAT, D_MODEL], BF16, isOutput=False)
    maskf = nc.declare_dram_parameter("maskf", [kpad], F32, isOutput=False)
    ident = nc.declare_dram_parameter("ident", [128, 128], F32, isOutput=False)
    ones8 = nc.declare_dram_parameter("ones8", [128, HPC], BF16, isOutput=False)
    y = nc.declare_dram_parameter("y", [T, D_MODEL], F32, isOutput=True)

    with ExitStack() as ctx:
        tc = ctx.enter_context(tile.TileContext(nc))

        # ---- long-lived pools (entered first so short-lived ones stack on top)
        const = ctx.enter_context(tc.tile_pool(name="const", bufs=1))
        qk_pool = ctx.enter_context(tc.tile_pool(name="qk", bufs=1))
        vaug_pool = ctx.enter_context(tc.tile_pool(name="vaug", bufs=1))

        identity = const.tile([128, 128], F32)
        nc.sync.dma_start(out=identity, in_=ident[:, :])

        # per-key-block additive exp bias: 0 for kept keys, -8e9 for pads
        maskb = const.tile([128, NKB], F32)
        nc.sync.dma_start(out=maskb, in_=maskf.rearrange("(j p) -> p j", p=128))

        # persistent intermeds
        # q/k of 2 heads per tile: rows [h0 d64 | h1 d64]
        qts = [qk_pool.tile([128, T], BF16, tag=f"qt{j}", name=f"qt{j}")
               for j in range(NPAIR)]
        kts = [qk_pool.tile([128, kpad], BF16, tag=f"kt{j}", name=f"kt{j}")
               for j in range(NPAIR)]
        # V per key-block: [128, head, 64].  Softmax denominators come
        # from separate M=1 ones-matmuls (col-tiled), not a ones column.
        vaugs = [vaug_pool.tile([128, HPC, D_HEAD], BF16,
                                tag=f"va{t}", name=f"va{t}")
                 for t in range(NKB)]
        onesd = const.tile([128, 1], BF16)
        nc.sync.dma_start(out=onesd, in_=ones8[:, 0:1])

        # alternate PSUM->SBUF copies between DVE and ACT so neither engine
        # serializes the PE in the projection phase
        _cp = [0]

        def copy_alt(out, in_):
            if _cp[0] % 2 == 0:
                nc.vector.tensor_copy(out=out, in_=in_)
            else:
                nc.scalar.activation(
                    out=out, in_=in_, func=mybir.ActivationFunctionType.Copy)
            _cp[0] += 1

        # x + projection weights stay resident for the whole kernel so the
        # q/k projections of pairs 1-3 can interleave into phase 2 (they are
        # the PE's filler work while ACT computes exp).
        xw_pool = ctx.enter_context(tc.tile_pool(name="xw", bufs=1))
        qkps = ctx.enter_context(tc.tile_pool(name="qkps", bufs=1, space="PSUM"))

        # ---- phase 1: V projection + pair-0 q/k projection
        with tc.tile_pool(name="vps", bufs=1, space="PSUM") as vps:
            # spread input DMAs over the three DMA-issuing engines (SP,
            # ACT, GPSIMD) so the first V-proj matmul only waits for
            # xk[0]+wv[0], not a 20us serial DMA chain
            xks = [xw_pool.tile([128, kpad], BF16, tag=f"xk{kc}", name=f"xk{kc}")
                   for kc in range(KC)]
            wvs = [xw_pool.tile([128, CAT], BF16, tag=f"wv{kc}", name=f"wv{kc}")
                   for kc in range(KC)]
            for kc in range(KC):
                nc.sync.dma_start(out=xks[kc],
                                  in_=xkT[kc * 128:(kc + 1) * 128, :])
                nc.sync.dma_start(out=wvs[kc],
                                  in_=wvT[kc * 128:(kc + 1) * 128, :])
            xqs = [xw_pool.tile([128, T], BF16, tag=f"xq{kc}", name=f"xq{kc}")
                   for kc in range(KC)]
            for kc in range(KC):
                nc.sync.dma_start(out=xqs[kc],
                                  in_=xT[kc * 128:(kc + 1) * 128, :])
            wqs = [xw_pool.tile([128, KC, 128], BF16, tag=f"wq{j}", name=f"wq{j}")
                   for j in range(NPAIR)]
            wks = [xw_pool.tile([128, KC, 128], BF16, tag=f"wk{j}", name=f"wk{j}")
                   for j in range(NPAIR)]
            for j in range(NPAIR):
                nc.scalar.dma_start(
                    out=wqs[j],
                    in_=wqT.rearrange("(kc p) c -> p kc c", p=128)[
                        :, :, j * 128:(j + 1) * 128])
                nc.scalar.dma_start(
                    out=wks[j],
                    in_=wkT.rearrange("(kc p) c -> p kc c", p=128)[
                        :, :, j * 128:(j + 1) * 128])

            # V projection over compacted keys: V[key, cat] = xk @ Wv^T.
            # kc-outer over groups of 7 live PSUM banks so compute starts
            # as soon as the first xk/wv tile pair lands.
            for g0 in range(0, NKB, 6):
                tbs = range(g0, min(g0 + 6, NKB))
                vp7 = {tb: vps.tile([128, CAT], F32, tag=f"vp{tb - g0}",
                                    name=f"vp{tb}") for tb in tbs}
                for kc in range(KC):
                    for tb in tbs:
                        nc.tensor.matmul(
                            vp7[tb],
                            xks[kc][:, tb * 128:(tb + 1) * 128],
                            wvs[kc],
                            start=(kc == 0), stop=(kc == KC - 1),
                        )
                for tb in tbs:
                    copy_alt(vaugs[tb],
                             vp7[tb].rearrange("p (h d) -> p h d", h=HPC))

            # Q projection (full T) and K projection (kpad), per head pair:
            # out rows = [q(2j) 64 | q(2j+1) 64] so one [128, chunk] copy
            # moves both heads at once.  Only pair 0 runs in phase 1; pairs
            # 1-3 are emitted chunk-by-chunk inside phase 2.
            def pair_chunks(j):
                return ([("q", j, c * 512, 512) for c in range(T // 512)]
                        + [("k", j, c0, w) for (c0, w) in KCH])

            def emit_chunk(spec, dve_only=False, pool=None, tag="qp"):
                kind, j, c0, w = spec
                qp = (pool or qkps).tile([128, 512], F32, tag=tag, name="qp")
                srcs = xqs if kind == "q" else xks
                wsrc = wqs[j] if kind == "q" else wks[j]
                dst = qts[j] if kind == "q" else kts[j]
                for kc in range(KC):
                    nc.tensor.matmul(
                        qp[:, 0:w],
                        wsrc[:, kc, :],
                        srcs[kc][:, c0:c0 + w],
                        start=(kc == 0), stop=(kc == KC - 1),
                    )
                if dve_only:
                    nc.vector.tensor_copy(out=dst[:, c0:c0 + w],
                                          in_=qp[:, 0:w])
                else:
                    copy_alt(dst[:, c0:c0 + w], qp[:, 0:w])

            for ci, spec in enumerate(pair_chunks(0)):
                if ci % 2 == 0:
                    emit_chunk(spec)
                else:
                    emit_chunk(spec, pool=vps, tag="qp1")

        # ---- phase 2: attention per head PAIR, q in four quarters.
        # The two heads of a pair live at SBUF partitions 0-63 / 64-127 of
        # qts/kts, so their K=64 S^T matmuls land on disjoint PE row groups
        # and execute CONCURRENTLY (measured 152ns vs 467ns per N=512 mm).
        # Both heads' scores for one (kb, quarter) go into one [128,2,512]
        # PSUM tile so a single 1024-wide exp covers them.
        num_pool = ctx.enter_context(tc.tile_pool(name="num", bufs=1))
        # stacked normalized attn^T: 2 heads per tile (cat rows)
        nums = [num_pool.tile([128, T], BF16, tag=f"nm{j}", name=f"nm{j}")
                for j in range(NPAIR)]
        NQTR = T // 512          # 4 q-quarters
        with tc.tile_pool(name="p_sb", bufs=5) as p_pool, \
             tc.tile_pool(name="av_sb", bufs=4) as avsb_pool, \
             tc.tile_pool(name="r_sb", bufs=4) as r_pool, \
             tc.tile_pool(name="at_sb", bufs=2) as at_pool, \
             tc.tile_pool(name="stps", bufs=2, space="PSUM") as stps, \
             tc.tile_pool(name="avps", bufs=1, space="PSUM") as avps, \
             tc.tile_pool(name="tps", bufs=1, space="PSUM") as tps:

            # deferred fine-grained PE work (normalize / flush), popped a
            # little per kb step so it never lumps into an ACT bubble
            deferred = []

            def pop_deferred(n, force=False):
                for _ in range(n):
                    if deferred and (force or len(deferred) > 2):
                        deferred.pop(0)()

            def norm_one(av_sb0, av_sb1, av_sbd, ap_tile, tb, i):
                # one token block: transpose the denominators (serving both
                # heads) and each head's attn, then scale per token
                td = tps.tile([128, 128], F32, tag="tp", name="td")
                nc.tensor.transpose(
                    td[:, 0:33],
                    av_sbd[:, i * 128:(i + 1) * 128],
                    identity[0:33, 0:33],
                )
                rs = []
                for hf, col in ((0, 0), (1, 32)):
                    r_sb = r_pool.tile([128, 1], F32, tag="r", name="r_sb")
                    nc.vector.reciprocal(out=r_sb, in_=td[:, col:col + 1])
                    rs.append(r_sb)
                for hf, av_sb in ((0, av_sb0), (1, av_sb1)):
                    t1 = tps.tile([128, 128], F32, tag="tp", name="t1")
                    nc.tensor.transpose(
                        t1[:, 0:D_HEAD],
                        av_sb[:, i * 128:(i + 1) * 128],
                        identity[0:D_HEAD, 0:D_HEAD],
                    )
                    nc.vector.tensor_scalar_mul(
                        ap_tile[:, tb, hf * 64:hf * 64 + 64],
                        t1[:, 0:D_HEAD], rs[hf])

            def flush_one(ap_tile, j, tb):
                t2 = tps.tile([128, 128], F32, tag="tp", name="t2")
                nc.tensor.transpose(t2, ap_tile[:, tb, :], identity)
                nc.vector.tensor_copy(
                    out=nums[j][:, tb * 128:(tb + 1) * 128], in_=t2)

            # projection work of upcoming pairs, flattened to single-mm ops
            # popped 2 per kb slot as PE filler
            chunk_ops = {}   # pair j -> list of closures

            def queue_chunk_ops(j):
                ops = []
                for spec in pair_chunks(j):
                    kind, jj, c0, w = spec
                    cell = {}
                    for kc in range(KC):
                        def mm(kc=kc, kind=kind, jj=jj, c0=c0, w=w, cell=cell):
                            if kc == 0:
                                cell["qp"] = qkps.tile([128, 512], F32,
                                                       tag="qp", name="qp")
                            srcs = xqs if kind == "q" else xks
                            wsrc = wqs[jj] if kind == "q" else wks[jj]
                            nc.tensor.matmul(
                                cell["qp"][:, 0:w],
                                wsrc[:, kc, :],
                                srcs[kc][:, c0:c0 + w],
                                start=(kc == 0), stop=(kc == KC - 1),
                            )
                        ops.append(mm)

                    def cp(kind=kind, jj=jj, c0=c0, w=w, cell=cell):
                        dst = qts[jj] if kind == "q" else kts[jj]
                        nc.vector.tensor_copy(out=dst[:, c0:c0 + w],
                                              in_=cell["qp"][:, 0:w])
                    ops.append(cp)
                chunk_ops[j] = ops

            def pop_chunk_ops(j, n):
                lst = chunk_ops.get(j)
                for _ in range(n):
                    if lst:
                        lst.pop(0)()

            for pair in range(NPAIR):
                # leftover projection work for THIS pair must finish now
                for op in chunk_ops.pop(pair, []):
                    op()
                if pair + 1 < NPAIR:
                    queue_chunk_ops(pair + 1)
                h0, h1 = 2 * pair, 2 * pair + 1
                # token-major normalized attn for the pair:
                # [tok-part, tok-blk, cat(2 heads x 64)]
                ap_tile = at_pool.tile([128, NTB, 128], F32,
                                       tag="ap", name="ap")
                avp_q = {}     # qtr -> (avp0, avp1)
                p_tiles = {}   # (qtr, kb) -> p2

                def emit_st_exp(qtr, kb, pair=pair):
                    q0 = qtr * 512
                    stp2 = stps.tile([128, 2, 512], F32, tag="stp",
                                     name="stp2")
                    nc.tensor.matmul(
                        stp2[:, 0, :],
                        kts[pair][0:64, kb * 128:(kb + 1) * 128],
                        qts[pair][0:64, q0:q0 + 512],
                        start=True, stop=True,
                    )
                    nc.tensor.matmul(
                        stp2[:, 1, :],
                        kts[pair][64:128, kb * 128:(kb + 1) * 128],
                        qts[pair][64:128, q0:q0 + 512],
                        start=True, stop=True,
                    )
                    p2 = p_pool.tile([128, 2, 512], BF16, tag="p",
                                     name="p2")
                    nc.scalar.activation(
                        out=p2, in_=stp2,
                        func=mybir.ActivationFunctionType.Exp,
                        bias=maskb[:, kb:kb + 1], scale=0.125,
                    )
                    p_tiles[(qtr, kb)] = p2

                def emit_av(qtr, kb, ap_tile=ap_tile):
                    if kb == 0:
                        avp_q[qtr] = (
                            avps.tile([128, 512], F32, tag="av2",
                                      name="avp2"),
                            avps.tile([33, 512], F32, tag="avd",
                                      name="avpd"),
                        )
                    avp2, avpd = avp_q[qtr]
                    p2 = p_tiles.pop((qtr, kb))
                    # both heads' attn col-tiled into one PSUM tile (PE col
                    # groups 0-1 / 2-3 -> concurrent), then both heads'
                    # denominators as M=1 col-tiled ones-matmuls
                    nc.tensor.matmul(
                        avp2[0:64, :], vaugs[kb][:, h0, :], p2[:, 0, :],
                        start=(kb == 0), stop=(kb == NKB - 1),
                        tile_position=(0, 0),
                    )
                    nc.tensor.matmul(
                        avp2[64:128, :], vaugs[kb][:, h1, :], p2[:, 1, :],
                        start=(kb == 0), stop=(kb == NKB - 1),
                        tile_position=(0, 64),
                    )
                    nc.tensor.matmul(
                        avpd[0:1, :], onesd, p2[:, 0, :],
                        start=(kb == 0), stop=(kb == NKB - 1),
                        tile_position=(0, 0),
                    )
                    nc.tensor.matmul(
                        avpd[32:33, :], onesd, p2[:, 1, :],
                        start=(kb == 0), stop=(kb == NKB - 1),
                        tile_position=(0, 32),
                    )
                    if kb == NKB - 1:
                        # drain the accumulators and queue fine-grained
                        # normalize work
                        av_sb0 = avsb_pool.tile([64, 512], F32, tag="avsb",
                                                name="av_sb0")
                        nc.vector.tensor_copy(out=av_sb0, in_=avp2[0:64, :])
                        av_sb1 = avsb_pool.tile([64, 512], F32, tag="avsb",
                                                name="av_sb1")
                        nc.vector.tensor_copy(out=av_sb1, in_=avp2[64:128, :])
                        av_sbd = avsb_pool.tile([33, 512], F32, tag="avsd",
                                                name="av_sbd")
                        nc.vector.tensor_copy(out=av_sbd, in_=avpd)
                        for i in range(4):
                            deferred.append(
                                lambda av_sb0=av_sb0, av_sb1=av_sb1,
                                av_sbd=av_sbd, i=i, tb=qtr * 4 + i:
                                norm_one(av_sb0, av_sb1, av_sbd,
                                         ap_tile, tb, i))
                        del avp_q[qtr]

                # one continuous kb stream across the pair's 4 quarters:
                # the ST->exp->AV lag spans quarter boundaries so ACT never
                # drains at a boundary
                steps = [(q, k) for q in range(NQTR) for k in range(NKB)]
                for i, (qtr, kb) in enumerate(steps):
                    emit_st_exp(qtr, kb)
                    if i >= LAG:
                        emit_av(*steps[i - LAG])
                    pop_chunk_ops(pair + 1, 2)
                    pop_deferred(2)
                for i in range(len(steps) - LAG, len(steps)):
                    emit_av(*steps[i])
                    pop_chunk_ops(pair + 1, 2)
                    pop_deferred(2)
                # queue this pair's flush for execution during the next pair
                for tb in range(NTB):
                    deferred.append(
                        lambda ap_tile=ap_tile, j=pair, tb=tb:
                        flush_one(ap_tile, j, tb))
            # drain the pipeline
            while deferred:
                pop_deferred(1, force=True)

        # ---- phase 3: tail matmul  y[tok, out] = attn_cat @ wtailT
        with tc.tile_pool(name="wt", bufs=1) as wt_pool, \
             tc.tile_pool(name="y_sb", bufs=3) as y_pool, \
             tc.tile_pool(name="yps", bufs=2, space="PSUM") as yps, \
             tc.tile_pool(name="dps3", bufs=1, space="PSUM") as dps3:

            def warm_keeper3():
                dmy3 = dps3.tile([128, 128], F32, tag="dmy3", name="dmy3")
                nc.tensor.matmul(dmy3, identity, identity, start=True, stop=True)
            wts = [wt_pool.tile([128, D_MODEL], BF16, tag=f"wt{c}", name=f"wt{c}")
                   for c in range(CAT // 128)]
            for c in range(CAT // 128):
                nc.sync.dma_start(out=wts[c], in_=wtailT[c * 128:(c + 1) * 128, :])
            for tb in range(NTB):
                warm_keeper3()
                yp = yps.tile([128, D_MODEL], F32, tag="yp")
                for n in range(D_MODEL // 512):
                    for c in range(CAT // 128):
                        nc.tensor.matmul(
                            yp[:, n * 512:(n + 1) * 512],
                            nums[c][:, tb * 128:(tb + 1) * 128],
                            wts[c][:, n * 512:(n + 1) * 512],
                            start=(c == 0), stop=(c == CAT // 128 - 1),
                        )
                y_sb = y_pool.tile([128, D_MODEL], F32, tag="ys")
                copy_alt(y_sb, yp)
                nc.sync.dma_start(out=y[tb * 128:(tb + 1) * 128, :], in_=y_sb)

    if split_waits:
        split_excess_waits(nc)
    return nc


_NC_CACHE = {}


def _get_nc(kpad):
    if kpad not in _NC_CACHE:
        _NC_CACHE[kpad] = build_nc(kpad)
    return _NC_CACHE[kpad]


def _plan(x, mask, w_qkv, w_tail):
    """Compute KPAD from the mask and shard full inputs into 8 core maps."""
    bf = mybir.dt.np(BF16)
    x = np.asarray(x, dtype=np.float32)
    mask = np.asarray(mask, dtype=np.int32)
    w_qkv = np.asarray(w_qkv, dtype=np.float32)
    w_tail = np.asarray(w_tail, dtype=np.float32)

    idxs = [np.flatnonzero(mask[b]) for b in range(BN)]
    nk_max = max(len(i) for i in idxs)
    kpad = max(128, -(-nk_max // 128) * 128)

    # per-batch compacted k/v-side inputs
    xTs, xkTs, maskfs = [], [], []
    for b in range(BN):
        idx = idxs[b]
        xkb = np.zeros((kpad, D_MODEL), dtype=np.float32)
        xkb[:len(idx)] = x[b][idx]
        mf = np.full(kpad, -8e9, dtype=np.float32)
        mf[:len(idx)] = 0.0
        xTs.append(np.ascontiguousarray(x[b].T).astype(bf))
        xkTs.append(np.ascontiguousarray(xkb.T).astype(bf))
        maskfs.append(mf)

    w3 = w_qkv.reshape(N_HEAD, 3, D_HEAD, D_MODEL)  # [head, qkv, d, dmodel]
    in_maps = []
    for c in range(8):
        b, hg = divmod(c, 2)
        H = range(hg * HPC, (hg + 1) * HPC)
        wq = np.concatenate([w3[h, 0] for h in H], axis=0)  # [512, 1024]
        wk = np.concatenate([w3[h, 1] for h in H], axis=0)
        wv = np.concatenate([w3[h, 2] for h in H], axis=0)
        wt = w_tail[:, hg * CAT:(hg + 1) * CAT]  # [1024, 512]
        in_maps.append({
            "ident": np.eye(128, dtype=np.float32),
            "ones8": np.ones((128, HPC), dtype=bf),
            "xT": xTs[b],
            "xkT": xkTs[b],
            "maskf": maskfs[b],
            "wqT": np.ascontiguousarray(wq.T).astype(bf),
            "wkT": np.ascontiguousarray(wk.T).astype(bf),
            "wvT": np.ascontiguousarray(wv.T).astype(bf),
            "wtailT": np.ascontiguousarray(wt.T).astype(bf),
        })
    return kpad, in_maps


def kernel(x, mask, w_qkv, w_tail, b_tail):
    kpad, in_maps = _plan(x, mask, w_qkv, w_tail)
    nc = _get_nc(kpad)
    last_err = None
    for _attempt in range(3):
        try:
            res = run_bass_kernel_spmd(nc, in_maps, list(range(8))).results
            break
        except Exception as e:  # transient device/runtime errors: retry
            last_err = e
            _time.sleep(3.0)
    else:
        raise last_err
    out = np.empty((BN, T, D_MODEL), dtype=np.float32)
    b_tail = np.asarray(b_tail, dtype=np.float32)
    for b in range(BN):
        out[b] = res[2 * b]["y"] + res[2 * b + 1]["y"] + b_tail
    return out


# revision 37
# speedup vs baseline: 1.3077x; 1.3077x over previous
"""Multi-head attention Trainium2 kernel, 8-way sharded, mask-compacted keys.

Problem: x[4,2048,1024] -> qkv proj (w_qkv [3072,1024]) -> 16-head attention
with key-padding mask -> tail proj (w_tail [1024,1024]) + b_tail.

Sharding: 8 shards = 4 batches x 2 head-groups (8 heads each). Each core
computes, for its (batch b, head-group hg):
  - q projection of x[b] (all T tokens) for its 8 heads
  - k/v projections of the mask-COMPACTED tokens of x[b] (keys with mask=0
    contribute exp(-inf)=0 to softmax, so they are dropped host-side and
    the key axis padded to KPAD, a multiple of 128; pads get bias -8e9)
  - [T x KPAD] masked attention per head
  - partial tail matmul y_part = attn_cat @ w_tail[:, cat_slice].T
Host unshards: out[b] = y_part[2b] + y_part[2b+1] + b_tail.  No collectives.

Engine strategy (trace-driven):
  - phase 1 (projections) and phase 3 (tail) are PE-dense: bf16 operands
    (1 cyc/row, background weight loads), PSUM->SBUF copies alternate
    between DVE and ACT so neither serializes the PE.
  - phase 2 (attention) is paced by ACT exp ([128,1024] tiles); PE operands
    stay float32r: the serial 4-byte weight load pads PE occupancy to
    ~match ACT, keeping the HAM clock at 8/8 (bf16 here made the PE idle
    23% per kb and the HAM halved the clock for ~180us).
"""

import time as _time

import numpy as np
from contextlib import ExitStack

import concourse.bass as bass
import concourse.mybir as mybir
import concourse.tile as tile
from concourse.bass_utils import run_bass_kernel_spmd

# ---------------------------------------------------------------------------
# walrus in this env accepts at most 2 sync waits per instruction; Tile's
# scheduler emits up to 10. Post-pass: peel excess waits onto same-engine
# NoOps inserted immediately before the offending instruction (same engine
# stream position => identical synchronization semantics).
MAX_WAITS = 1


def split_excess_waits(nc):
    for fn in nc.m.functions:
        for bb in fn.blocks:
            insts = list(bb.instructions)
            out = []
            changed = False
            for inst in insts:
                si = inst.sync_info
                waits = list(si.on_wait) if si is not None else []
                if len(waits) > MAX_WAITS:
                    extra = waits[:-MAX_WAITS]
                    for ci in range(0, len(extra), MAX_WAITS):
                        chunk = extra[ci:ci + MAX_WAITS]
                        nop = mybir.InstNoOp(
                            name=f"{inst.name}-ws{ci}", ins=[], outs=[])
                        nop.engine = inst.engine
                        nop.sync_info = mybir.SyncInfo(
                            on_wait=chunk, on_update=[])
                        out.append(nop)
                    inst.sync_info = mybir.SyncInfo(
                        on_wait=waits[-MAX_WAITS:],
                        on_update=list(si.on_update))
                    changed = True
                out.append(inst)
            if changed:
                bb.instructions = out
# ---------------------------------------------------------------------------

D_MODEL = 1024
N_HEAD = 16
D_HEAD = 64
BN, T = 4, 2048
HPC = 8                      # heads per core
NPAIR = HPC // 2             # head pairs (q/k tiles hold 2 heads)
CAT = HPC * D_HEAD           # 512 per-core tail contraction
NTB = T // 128               # 16 query-token blocks
QH = T // 2                  # 1024, q processed in two halves
KC = D_MODEL // 128          # 8 contraction chunks
F32 = mybir.dt.float32
F32R = mybir.dt.float32r
BF16 = mybir.dt.bfloat16
I32 = mybir.dt.int32


def build_nc(kpad, split_waits=True):
    assert kpad % 128 == 0 and 128 <= kpad <= T
    NKB = kpad // 128        # key blocks
    LAG = min(4, NKB - 1) if NKB > 1 else 0
    # k-projection chunks: (start, width), width 512 or the tail remainder
    KCH = [(c * 512, min(512, kpad - c * 512))
           for c in range((kpad + 511) // 512)]

    nc = bass.Bass()
    xT = nc.declare_dram_parameter("xT", [D_MODEL, T], BF16, isOutput=False)
    xkT = nc.declare_dram_parameter("xkT", [D_MODEL, kpad], BF16, isOutput=False)
    wqT = nc.declare_dram_parameter("wqT", [D_MODEL, CAT], BF16, isOutput=False)
    wkT = nc.declare_dram_parameter("wkT", [D_MODEL, CAT], BF16, isOutput=False)
    wvT = nc.declare_dram_parameter("wvT", [D_MODEL, CAT], BF16, isOutput=False)
    wtailT = nc.declare_dram_parameter("wtailT", [CAT, D_MODEL], BF16, isOutput=False)
    maskf = nc.declare_dram_parameter("maskf", [kpad], F32, isOutput=False)
    ident = nc.declare_dram_parameter("ident", [128, 128], F32, isOutput=False)
    ones8 = nc.declare_dram_parameter("ones8", [128, HPC], BF16, isOutput=False)
    y = nc.declare_dram_parameter("y", [T, D_MODEL], F32, isOutput=True)

    with ExitStack() as ctx:
        tc = ctx.enter_context(tile.TileContext(nc))

        # ---- long-lived pools (entered first so short-lived ones stack on top)
        const = ctx.enter_context(tc.tile_pool(name="const", bufs=1))
        qk_pool = ctx.enter_context(tc.tile_pool(name="qk", bufs=1))
        vaug_pool = ctx.enter_context(tc.tile_pool(name="vaug", bufs=1))

        identity = const.tile([128, 128], F32)
        nc.sync.dma_start(out=identity, in_=ident[:, :])

        # per-key-block additive exp bias: 0 for kept keys, -8e9 for pads
        maskb = const.tile([128, NKB], F32)
        nc.sync.dma_start(out=maskb, in_=maskf.rearrange("(j p) -> p j", p=128))

        # persistent intermeds
        # q/k of 2 heads per tile: rows [h0 d64 | h1 d64]
        qts = [qk_pool.tile([128, T], BF16, tag=f"qt{j}", name=f"qt{j}")
               for j in range(NPAIR)]
        kts = [qk_pool.tile([128, kpad], BF16, tag=f"kt{j}", name=f"kt{j}")
               for j in range(NPAIR)]
        # V augmented with ones column: [key-block][128, head, 65]
        vaugs = [vaug_pool.tile([128, HPC, D_HEAD + 1], BF16,
                                tag=f"va{t}", name=f"va{t}")
                 for t in range(NKB)]

        # alternate PSUM->SBUF copies between DVE and ACT so neither engine
        # serializes the PE in the projection phase
        _cp = [0]

        def copy_alt(out, in_):
            if _cp[0] % 2 == 0:
                nc.vector.tensor_copy(out=out, in_=in_)
            else:
                nc.scalar.activation(
                    out=out, in_=in_, func=mybir.ActivationFunctionType.Copy)
            _cp[0] += 1

        # x + projection weights stay resident for the whole kernel so the
        # q/k projections of pairs 1-3 can interleave into phase 2 (they are
        # the PE's filler work while ACT computes exp).
        xw_pool = ctx.enter_context(tc.tile_pool(name="xw", bufs=1))
        qkps = ctx.enter_context(tc.tile_pool(name="qkps", bufs=1, space="PSUM"))

        # ---- phase 1: V projection + pair-0 q/k projection
        with tc.tile_pool(name="vps", bufs=1, space="PSUM") as vps:
            # spread input DMAs over the three DMA-issuing engines (SP,
            # ACT, GPSIMD) so the first V-proj matmul only waits for
            # xk[0]+wv[0], not a 20us serial DMA chain
            xks = [xw_pool.tile([128, kpad], BF16, tag=f"xk{kc}", name=f"xk{kc}")
                   for kc in range(KC)]
            wvs = [xw_pool.tile([128, CAT], BF16, tag=f"wv{kc}", name=f"wv{kc}")
                   for kc in range(KC)]
            for kc in range(KC):
                nc.sync.dma_start(out=xks[kc],
                                  in_=xkT[kc * 128:(kc + 1) * 128, :])
                nc.sync.dma_start(out=wvs[kc],
                                  in_=wvT[kc * 128:(kc + 1) * 128, :])
            xqs = [xw_pool.tile([128, T], BF16, tag=f"xq{kc}", name=f"xq{kc}")
                   for kc in range(KC)]
            for kc in range(KC):
                nc.sync.dma_start(out=xqs[kc],
                                  in_=xT[kc * 128:(kc + 1) * 128, :])
            wqs = [xw_pool.tile([128, KC, 128], BF16, tag=f"wq{j}", name=f"wq{j}")
                   for j in range(NPAIR)]
            wks = [xw_pool.tile([128, KC, 128], BF16, tag=f"wk{j}", name=f"wk{j}")
                   for j in range(NPAIR)]
            for j in range(NPAIR):
                nc.scalar.dma_start(
                    out=wqs[j],
                    in_=wqT.rearrange("(kc p) c -> p kc c", p=128)[
                        :, :, j * 128:(j + 1) * 128])
                nc.scalar.dma_start(
                    out=wks[j],
                    in_=wkT.rearrange("(kc p) c -> p kc c", p=128)[
                        :, :, j * 128:(j + 1) * 128])

            # V projection over compacted keys: V[key, cat] = xk @ Wv^T.
            # kc-outer over groups of 7 live PSUM banks so compute starts
            # as soon as the first xk/wv tile pair lands.
            for g0 in range(0, NKB, 6):
                tbs = range(g0, min(g0 + 6, NKB))
                vp7 = {tb: vps.tile([128, CAT], F32, tag=f"vp{tb - g0}",
                                    name=f"vp{tb}") for tb in tbs}
                for kc in range(KC):
                    for tb in tbs:
                        nc.tensor.matmul(
                            vp7[tb],
                            xks[kc][:, tb * 128:(tb + 1) * 128],
                            wvs[kc],
                            start=(kc == 0), stop=(kc == KC - 1),
                        )
                for tb in tbs:
                    va = vaugs[tb]
                    nc.sync.dma_start(
                        out=va[:, :, D_HEAD:D_HEAD + 1], in_=ones8[:, :])
                    copy_alt(va[:, :, 0:D_HEAD],
                             vp7[tb].rearrange("p (h d) -> p h d", h=HPC))

            # Q projection (full T) and K projection (kpad), per head pair:
            # out rows = [q(2j) 64 | q(2j+1) 64] so one [128, chunk] copy
            # moves both heads at once.  Only pair 0 runs in phase 1; pairs
            # 1-3 are emitted chunk-by-chunk inside phase 2.
            def pair_chunks(j):
                return ([("q", j, c * 512, 512) for c in range(T // 512)]
                        + [("k", j, c0, w) for (c0, w) in KCH])

            def emit_chunk(spec, dve_only=False, pool=None, tag="qp"):
                kind, j, c0, w = spec
                qp = (pool or qkps).tile([128, 512], F32, tag=tag, name="qp")
                srcs = xqs if kind == "q" else xks
                wsrc = wqs[j] if kind == "q" else wks[j]
                dst = qts[j] if kind == "q" else kts[j]
                for kc in range(KC):
                    nc.tensor.matmul(
                        qp[:, 0:w],
                        wsrc[:, kc, :],
                        srcs[kc][:, c0:c0 + w],
                        start=(kc == 0), stop=(kc == KC - 1),
                    )
                if dve_only:
                    nc.vector.tensor_copy(out=dst[:, c0:c0 + w],
                                          in_=qp[:, 0:w])
                else:
                    copy_alt(dst[:, c0:c0 + w], qp[:, 0:w])

            for ci, spec in enumerate(pair_chunks(0)):
                if ci % 2 == 0:
                    emit_chunk(spec)
                else:
                    emit_chunk(spec, pool=vps, tag="qp1")

        # ---- phase 2: attention per head PAIR, q in four quarters.
        # The two heads of a pair live at SBUF partitions 0-63 / 64-127 of
        # qts/kts, so their K=64 S^T matmuls land on disjoint PE row groups
        # and execute CONCURRENTLY (measured 152ns vs 467ns per N=512 mm).
        # Both heads' scores for one (kb, quarter) go into one [128,2,512]
        # PSUM tile so a single 1024-wide exp covers them.
        num_pool = ctx.enter_context(tc.tile_pool(name="num", bufs=1))
        # stacked normalized attn^T: 2 heads per tile (cat rows)
        nums = [num_pool.tile([128, T], BF16, tag=f"nm{j}", name=f"nm{j}")
                for j in range(NPAIR)]
        NQTR = T // 512          # 4 q-quarters
        with tc.tile_pool(name="p_sb", bufs=5) as p_pool, \
             tc.tile_pool(name="av_sb", bufs=4) as avsb_pool, \
             tc.tile_pool(name="r_sb", bufs=4) as r_pool, \
             tc.tile_pool(name="at_sb", bufs=2) as at_pool, \
             tc.tile_pool(name="stps", bufs=2, space="PSUM") as stps, \
             tc.tile_pool(name="avps", bufs=1, space="PSUM") as avps, \
             tc.tile_pool(name="tps", bufs=1, space="PSUM") as tps:

            # deferred fine-grained PE work (normalize / flush), popped a
            # little per kb step so it never lumps into an ACT bubble
            deferred = []

            def pop_deferred(n, force=False):
                for _ in range(n):
                    if deferred and (force or len(deferred) > 2):
                        deferred.pop(0)()

            def norm_one(av_sb, ap_tile, r0, tb, i):
                t1 = tps.tile([128, 128], F32, tag="tp", name="t1")
                nc.tensor.transpose(
                    t1[:, 0:D_HEAD + 1],
                    av_sb[:, i * 128:(i + 1) * 128],
                    identity[0:D_HEAD + 1, 0:D_HEAD + 1],
                )
                r_sb = r_pool.tile([128, 1], F32, tag="r", name="r_sb")
                nc.vector.reciprocal(out=r_sb, in_=t1[:, D_HEAD:D_HEAD + 1])
                nc.vector.tensor_scalar_mul(
                    ap_tile[:, tb, r0:r0 + 64], t1[:, 0:D_HEAD], r_sb)

            def flush_one(ap_tile, j, tb):
                t2 = tps.tile([128, 128], F32, tag="tp", name="t2")
                nc.tensor.transpose(t2, ap_tile[:, tb, :], identity)
                nc.vector.tensor_copy(
                    out=nums[j][:, tb * 128:(tb + 1) * 128], in_=t2)

            # projection work of upcoming pairs, flattened to single-mm ops
            # popped 2 per kb slot as PE filler
            chunk_ops = {}   # pair j -> list of closures

            def queue_chunk_ops(j):
                ops = []
                for spec in pair_chunks(j):
                    kind, jj, c0, w = spec
                    cell = {}
                    for kc in range(KC):
                        def mm(kc=kc, kind=kind, jj=jj, c0=c0, w=w, cell=cell):
                            if kc == 0:
                                cell["qp"] = qkps.tile([128, 512], F32,
                                                       tag="qp", name="qp")
                            srcs = xqs if kind == "q" else xks
                            wsrc = wqs[jj] if kind == "q" else wks[jj]
                            nc.tensor.matmul(
                                cell["qp"][:, 0:w],
                                wsrc[:, kc, :],
                                srcs[kc][:, c0:c0 + w],
                                start=(kc == 0), stop=(kc == KC - 1),
                            )
                        ops.append(mm)

                    def cp(kind=kind, jj=jj, c0=c0, w=w, cell=cell):
                        dst = qts[jj] if kind == "q" else kts[jj]
                        nc.vector.tensor_copy(out=dst[:, c0:c0 + w],
                                              in_=cell["qp"][:, 0:w])
                    ops.append(cp)
                chunk_ops[j] = ops

            def pop_chunk_ops(j, n):
                lst = chunk_ops.get(j)
                for _ in range(n):
                    if lst:
                        lst.pop(0)()

            for pair in range(NPAIR):
                # leftover projection work for THIS pair must finish now
                for op in chunk_ops.pop(pair, []):
                    op()
                if pair + 1 < NPAIR:
                    queue_chunk_ops(pair + 1)
                h0, h1 = 2 * pair, 2 * pair + 1
                # token-major normalized attn for the pair:
                # [tok-part, tok-blk, cat(2 heads x 64)]
                ap_tile = at_pool.tile([128, NTB, 128], F32,
                                       tag="ap", name="ap")
                avp_q = {}     # qtr -> (avp0, avp1)
                p_tiles = {}   # (qtr, kb) -> p2

                def emit_st_exp(qtr, kb, pair=pair):
                    q0 = qtr * 512
                    stp2 = stps.tile([128, 2, 512], F32, tag="stp",
                                     name="stp2")
                    nc.tensor.matmul(
                        stp2[:, 0, :],
                        kts[pair][0:64, kb * 128:(kb + 1) * 128],
                        qts[pair][0:64, q0:q0 + 512],
                        start=True, stop=True,
                    )
                    nc.tensor.matmul(
                        stp2[:, 1, :],
                        kts[pair][64:128, kb * 128:(kb + 1) * 128],
                        qts[pair][64:128, q0:q0 + 512],
                        start=True, stop=True,
                    )
                    p2 = p_pool.tile([128, 2, 512], BF16, tag="p",
                                     name="p2")
                    nc.scalar.activation(
                        out=p2, in_=stp2,
                        func=mybir.ActivationFunctionType.Exp,
                        bias=maskb[:, kb:kb + 1], scale=0.125,
                    )
                    p_tiles[(qtr, kb)] = p2

                def emit_av(qtr, kb, ap_tile=ap_tile):
                    if kb == 0:
                        avp_q[qtr] = (
                            avps.tile([D_HEAD + 1, 512], F32, tag="av0",
                                      name="avp0"),
                            avps.tile([D_HEAD + 1, 512], F32, tag="av1",
                                      name="avp1"),
                        )
                    avp0, avp1 = avp_q[qtr]
                    p2 = p_tiles.pop((qtr, kb))
                    nc.tensor.matmul(
                        avp0, vaugs[kb][:, h0, :], p2[:, 0, :],
                        start=(kb == 0), stop=(kb == NKB - 1),
                    )
                    nc.tensor.matmul(
                        avp1, vaugs[kb][:, h1, :], p2[:, 1, :],
                        start=(kb == 0), stop=(kb == NKB - 1),
                    )
                    if kb == NKB - 1:
                        # drain the accumulators and queue fine-grained
                        # normalize work
                        for r0, avp in ((0, avp0), (64, avp1)):
                            av_sb = avsb_pool.tile(
                                [D_HEAD + 1, 512], F32, tag="avsb",
                                name="av_sb")
                            nc.vector.tensor_copy(out=av_sb, in_=avp)
                            for i in range(4):
                                deferred.append(
                                    lambda av_sb=av_sb, r0=r0, i=i,
                                    tb=qtr * 4 + i:
                                    norm_one(av_sb, ap_tile, r0, tb, i))
                        del avp_q[qtr]

                # one continuous kb stream across the pair's 4 quarters:
                # the ST->exp->AV lag spans quarter boundaries so ACT never
                # drains at a boundary
                steps = [(q, k) for q in range(NQTR) for k in range(NKB)]
                for i, (qtr, kb) in enumerate(steps):
                    emit_st_exp(qtr, kb)
                    if i >= LAG:
                        emit_av(*steps[i - LAG])
                    pop_chunk_ops(pair + 1, 2)
                    pop_deferred(2)
                for i in range(len(steps) - LAG, len(steps)):
                    emit_av(*steps[i])
                    pop_chunk_ops(pair + 1, 2)
                    pop_deferred(2)
                # queue this pair's flush for execution during the next pair
                for tb in range(NTB):
                    deferred.append(
                        lambda ap_tile=ap_tile, j=pair, tb=tb:
                        flush_one(ap_tile, j, tb))
            # drain the pipeline
            while deferred:
                pop_deferred(1, force=True)

        # ---- phase 3: tail matmul  y[tok, out] = attn_cat @ wtailT
        with tc.tile_pool(name="wt", bufs=1) as wt_pool, \
             tc.tile_pool(name="y_sb", bufs=3) as y_pool, \
             tc.tile_pool(name="yps", bufs=2, space="PSUM") as yps, \
             tc.tile_pool(name="dps3", bufs=1, space="PSUM") as dps3:

            def warm_keeper3():
                dmy3 = dps3.tile([128, 128], F32, tag="dmy3", name="dmy3")
                nc.tensor.matmul(dmy3, identity, identity, start=True, stop=True)
            wts = [wt_pool.tile([128, D_MODEL], BF16, tag=f"wt{c}", name=f"wt{c}")
                   for c in range(CAT // 128)]
            for c in range(CAT // 128):
                nc.sync.dma_start(out=wts[c], in_=wtailT[c * 128:(c + 1) * 128, :])
            for tb in range(NTB):
                warm_keeper3()
                yp = yps.tile([128, D_MODEL], F32, tag="yp")
                for n in range(D_MODEL // 512):
                    for c in range(CAT // 128):
                        nc.tensor.matmul(
                            yp[:, n * 512:(n + 1) * 512],
                            nums[c][:, tb * 128:(tb + 1) * 128],
                            wts[c][:, n * 512:(n + 1) * 512],
                            start=(c == 0), stop=(c == CAT // 128 - 1),
                        )
                y_sb = y_pool.tile([128, D_MODEL], F32, tag="ys")
                copy_alt(y_sb, yp)
                nc.sync.dma_start(out=y[tb * 128:(tb + 1) * 128, :], in_=y_sb)

    if split_waits:
        split_excess_waits(nc)
    return nc


_NC_CACHE = {}


def _get_nc(kpad):
    if kpad not in _NC_CACHE:
        _NC_CACHE[kpad] = build_nc(kpad)
    return _NC_CACHE[kpad]


def _plan(x, mask, w_qkv, w_tail):
    """Compute KPAD from the mask and shard full inputs into 8 core maps."""
    bf = mybir.dt.np(BF16)
    x = np.asarray(x, dtype=np.float32)
    mask = np.asarray(mask, dtype=np.int32)
    w_qkv = np.asarray(w_qkv, dtype=np.float32)
    w_tail = np.asarray(w_tail, dtype=np.float32)

    idxs = [np.flatnonzero(mask[b]) for b in range(BN)]
    nk_max = max(len(i) for i in idxs)
    kpad = max(128, -(-nk_max // 128) * 128)

    # per-batch compacted k/v-side inputs
    xTs, xkTs, maskfs = [], [], []
    for b in range(BN):
        idx = idxs[b]
        xkb = np.zeros((kpad, D_MODEL), dtype=np.float32)
        xkb[:len(idx)] = x[b][idx]
        mf = np.full(kpad, -8e9, dtype=np.float32)
        mf[:len(idx)] = 0.0
        xTs.append(np.ascontiguousarray(x[b].T).astype(bf))
        xkTs.append(np.ascontiguousarray(xkb.T).astype(bf))
        maskfs.append(mf)

    w3 = w_qkv.reshape(N_HEAD, 3, D_HEAD, D_MODEL)  # [head, qkv, d, dmodel]
    in_maps = []
    for c in range(8):
        b, hg = divmod(c, 2)
        H = range(hg * HPC, (hg + 1) * HPC)
        wq = np.concatenate([w3[h, 0] for h in H], axis=0)  # [512, 1024]
        wk = np.concatenate([w3[h, 1] for h in H], axis=0)
        wv = np.concatenate([w3[h, 2] for h in H], axis=0)
        wt = w_tail[:, hg * CAT:(hg + 1) * CAT]  # [1024, 512]
        in_maps.append({
            "ident": np.eye(128, dtype=np.float32),
            "ones8": np.ones((128, HPC), dtype=bf),
            "xT": xTs[b],
            "xkT": xkTs[b],
            "maskf": maskfs[b],
            "wqT": np.ascontiguousarray(wq.T).astype(bf),
            "wkT": np.ascontiguousarray(wk.T).astype(bf),
            "wvT": np.ascontiguousarray(wv.T).astype(bf),
            "wtailT": np.ascontiguousarray(wt.T).astype(bf),
        })
    return kpad, in_maps


def kernel(x, mask, w_qkv, w_tail, b_tail):
    kpad, in_maps = _plan(x, mask, w_qkv, w_tail)
    nc = _get_nc(kpad)
    last_err = None
    for _attempt in range(3):
        try:
            res = run_bass_kernel_spmd(nc, in_maps, list(range(8))).results
            break
        except Exception as e:  # transient device/runtime errors: retry
            last_err = e
            _time.sleep(3.0)
    else:
        raise last_err
    out = np.empty((BN, T, D_MODEL), dtype=np.float32)
    b_tail = np.asarray(b_tail, dtype=np.float32)
    for b in range(BN):
        out[b] = res[2 * b]["y"] + res[2 * b + 1]["y"] + b_tail
    return out


# revision 38
# speedup vs baseline: 1.3131x; 1.0041x over previous
"""Multi-head attention Trainium2 kernel, 8-way sharded, mask-compacted keys.

Problem: x[4,2048,1024] -> qkv proj (w_qkv [3072,1024]) -> 16-head attention
with key-padding mask -> tail proj (w_tail [1024,1024]) + b_tail.

Sharding: 8 shards = 4 batches x 2 head-groups (8 heads each). Each core
computes, for its (batch b, head-group hg):
  - q projection of x[b] (all T tokens) for its 8 heads
  - k/v projections of the mask-COMPACTED tokens of x[b] (keys with mask=0
    contribute exp(-inf)=0 to softmax, so they are dropped host-side and
    the key axis padded to KPAD, a multiple of 128; pads get bias -8e9)
  - [T x KPAD] masked attention per head
  - partial tail matmul y_part = attn_cat @ w_tail[:, cat_slice].T
Host unshards: out[b] = y_part[2b] + y_part[2b+1] + b_tail.  No collectives.

Engine strategy (trace-driven):
  - phase 1 (projections) and phase 3 (tail) are PE-dense: bf16 operands
    (1 cyc/row, background weight loads), PSUM->SBUF copies alternate
    between DVE and ACT so neither serializes the PE.
  - phase 2 (attention) is paced by ACT exp ([128,1024] tiles); PE operands
    stay float32r: the serial 4-byte weight load pads PE occupancy to
    ~match ACT, keeping the HAM clock at 8/8 (bf16 here made the PE idle
    23% per kb and the HAM halved the clock for ~180us).
"""

import time as _time

import numpy as np
from contextlib import ExitStack

import concourse.bass as bass
import concourse.mybir as mybir
import concourse.tile as tile
from concourse.bass_utils import run_bass_kernel_spmd

# ---------------------------------------------------------------------------
# walrus in this env accepts at most 2 sync waits per instruction; Tile's
# scheduler emits up to 10. Post-pass: peel excess waits onto same-engine
# NoOps inserted immediately before the offending instruction (same engine
# stream position => identical synchronization semantics).
MAX_WAITS = 1


def split_excess_waits(nc):
    for fn in nc.m.functions:
        for bb in fn.blocks:
            insts = list(bb.instructions)
            out = []
            changed = False
            for inst in insts:
                si = inst.sync_info
                waits = list(si.on_wait) if si is not None else []
                if len(waits) > MAX_WAITS:
                    extra = waits[:-MAX_WAITS]
                    for ci in range(0, len(extra), MAX_WAITS):
                        chunk = extra[ci:ci + MAX_WAITS]
                        nop = mybir.InstNoOp(
                            name=f"{inst.name}-ws{ci}", ins=[], outs=[])
                        nop.engine = inst.engine
                        nop.sync_info = mybir.SyncInfo(
                            on_wait=chunk, on_update=[])
                        out.append(nop)
                    inst.sync_info = mybir.SyncInfo(
                        on_wait=waits[-MAX_WAITS:],
                        on_update=list(si.on_update))
                    changed = True
                out.append(inst)
            if changed:
                bb.instructions = out
# ---------------------------------------------------------------------------

D_MODEL = 1024
N_HEAD = 16
D_HEAD = 64
BN, T = 4, 2048
HPC = 8                      # heads per core
NPAIR = HPC // 2             # head pairs (q/k tiles hold 2 heads)
CAT = HPC * D_HEAD           # 512 per-core tail contraction
NTB = T // 128               # 16 query-token blocks
QH = T // 2                  # 1024, q processed in two halves
KC = D_MODEL // 128          # 8 contraction chunks
F32 = mybir.dt.float32
F32R = mybir.dt.float32r
BF16 = mybir.dt.bfloat16
I32 = mybir.dt.int32


def build_nc(kpad, split_waits=True):
    assert kpad % 128 == 0 and 128 <= kpad <= T
    NKB = kpad // 128        # key blocks
    LAG = min(4, NKB - 1) if NKB > 1 else 0
    # k-projection chunks: (start, width), width 512 or the tail remainder
    KCH = [(c * 512, min(512, kpad - c * 512))
           for c in range((kpad + 511) // 512)]

    nc = bass.Bass()
    xT = nc.declare_dram_parameter("xT", [D_MODEL, T], BF16, isOutput=False)
    xkT = nc.declare_dram_parameter("xkT", [D_MODEL, kpad], BF16, isOutput=False)
    wqT = nc.declare_dram_parameter("wqT", [D_MODEL, CAT], BF16, isOutput=False)
    wkT = nc.declare_dram_parameter("wkT", [D_MODEL, CAT], BF16, isOutput=False)
    wvT = nc.declare_dram_parameter("wvT", [D_MODEL, CAT], BF16, isOutput=False)
    wtailT = nc.declare_dram_parameter("wtailT", [CAT, D_MODEL], BF16, isOutput=False)
    maskf = nc.declare_dram_parameter("maskf", [kpad], F32, isOutput=False)
    ident = nc.declare_dram_parameter("ident", [128, 128], F32, isOutput=False)
    ones8 = nc.declare_dram_parameter("ones8", [128, HPC], BF16, isOutput=False)
    y = nc.declare_dram_parameter("y", [T, D_MODEL], F32, isOutput=True)

    with ExitStack() as ctx:
        tc = ctx.enter_context(tile.TileContext(nc))

        # ---- long-lived pools (entered first so short-lived ones stack on top)
        const = ctx.enter_context(tc.tile_pool(name="const", bufs=1))
        qk_pool = ctx.enter_context(tc.tile_pool(name="qk", bufs=1))
        vaug_pool = ctx.enter_context(tc.tile_pool(name="vaug", bufs=1))

        identity = const.tile([128, 128], F32)
        nc.sync.dma_start(out=identity, in_=ident[:, :])

        # per-key-block additive exp bias: 0 for kept keys, -8e9 for pads
        maskb = const.tile([128, NKB], F32)
        nc.sync.dma_start(out=maskb, in_=maskf.rearrange("(j p) -> p j", p=128))

        # persistent intermeds
        # q/k of 2 heads per tile: rows [h0 d64 | h1 d64]
        qts = [qk_pool.tile([128, T], BF16, tag=f"qt{j}", name=f"qt{j}")
               for j in range(NPAIR)]
        kts = [qk_pool.tile([128, kpad], BF16, tag=f"kt{j}", name=f"kt{j}")
               for j in range(NPAIR)]
        # V augmented with ones column: [key-block][128, head, 65]
        vaugs = [vaug_pool.tile([128, HPC, D_HEAD + 1], BF16,
                                tag=f"va{t}", name=f"va{t}")
                 for t in range(NKB)]

        # alternate PSUM->SBUF copies between DVE and ACT so neither engine
        # serializes the PE in the projection phase
        _cp = [0]

        def copy_alt(out, in_):
            if _cp[0] % 2 == 0:
                nc.vector.tensor_copy(out=out, in_=in_)
            else:
                nc.scalar.activation(
                    out=out, in_=in_, func=mybir.ActivationFunctionType.Copy)
            _cp[0] += 1

        # x + projection weights stay resident for the whole kernel so the
        # q/k projections of pairs 1-3 can interleave into phase 2 (they are
        # the PE's filler work while ACT computes exp).
        xw_pool = ctx.enter_context(tc.tile_pool(name="xw", bufs=1))
        qkps = ctx.enter_context(tc.tile_pool(name="qkps", bufs=1, space="PSUM"))

        # ---- phase 1: V projection + pair-0 q/k projection
        with tc.tile_pool(name="vps", bufs=1, space="PSUM") as vps:
            # spread input DMAs over the three DMA-issuing engines (SP,
            # ACT, GPSIMD) so the first V-proj matmul only waits for
            # xk[0]+wv[0], not a 20us serial DMA chain
            xks = [xw_pool.tile([128, kpad], BF16, tag=f"xk{kc}", name=f"xk{kc}")
                   for kc in range(KC)]
            wvs = [xw_pool.tile([128, CAT], BF16, tag=f"wv{kc}", name=f"wv{kc}")
                   for kc in range(KC)]
            for kc in range(KC):
                nc.sync.dma_start(out=xks[kc],
                                  in_=xkT[kc * 128:(kc + 1) * 128, :])
                nc.sync.dma_start(out=wvs[kc],
                                  in_=wvT[kc * 128:(kc + 1) * 128, :])
            xqs = [xw_pool.tile([128, T], BF16, tag=f"xq{kc}", name=f"xq{kc}")
                   for kc in range(KC)]
            for kc in range(KC):
                nc.sync.dma_start(out=xqs[kc],
                                  in_=xT[kc * 128:(kc + 1) * 128, :])
            wqs = [xw_pool.tile([128, KC, 128], BF16, tag=f"wq{j}", name=f"wq{j}")
                   for j in range(NPAIR)]
            wks = [xw_pool.tile([128, KC, 128], BF16, tag=f"wk{j}", name=f"wk{j}")
                   for j in range(NPAIR)]
            for j in range(NPAIR):
                nc.scalar.dma_start(
                    out=wqs[j],
                    in_=wqT.rearrange("(kc p) c -> p kc c", p=128)[
                        :, :, j * 128:(j + 1) * 128])
                nc.scalar.dma_start(
                    out=wks[j],
                    in_=wkT.rearrange("(kc p) c -> p kc c", p=128)[
                        :, :, j * 128:(j + 1) * 128])

            # V projection over compacted keys: V[key, cat] = xk @ Wv^T.
            # kc-outer over groups of 7 live PSUM banks so compute starts
            # as soon as the first xk/wv tile pair lands.
            for g0 in range(0, NKB, 6):
                tbs = range(g0, min(g0 + 6, NKB))
                vp7 = {tb: vps.tile([128, CAT], F32, tag=f"vp{tb - g0}",
                                    name=f"vp{tb}") for tb in tbs}
                for kc in range(KC):
                    for tb in tbs:
                        nc.tensor.matmul(
                            vp7[tb],
                            xks[kc][:, tb * 128:(tb + 1) * 128],
                            wvs[kc],
                            start=(kc == 0), stop=(kc == KC - 1),
                        )
                for tb in tbs:
                    va = vaugs[tb]
                    nc.sync.dma_start(
                        out=va[:, :, D_HEAD:D_HEAD + 1], in_=ones8[:, :])
                    copy_alt(va[:, :, 0:D_HEAD],
                             vp7[tb].rearrange("p (h d) -> p h d", h=HPC))

            # Q projection (full T) and K projection (kpad), per head pair:
            # out rows = [q(2j) 64 | q(2j+1) 64] so one [128, chunk] copy
            # moves both heads at once.  Only pair 0 runs in phase 1; pairs
            # 1-3 are emitted chunk-by-chunk inside phase 2.
            def pair_chunks(j):
                return ([("q", j, c * 512, 512) for c in range(T // 512)]
                        + [("k", j, c0, w) for (c0, w) in KCH])

            def emit_chunk(spec, dve_only=False, pool=None, tag="qp"):
                kind, j, c0, w = spec
                qp = (pool or qkps).tile([128, 512], F32, tag=tag, name="qp")
                srcs = xqs if kind == "q" else xks
                wsrc = wqs[j] if kind == "q" else wks[j]
                dst = qts[j] if kind == "q" else kts[j]
                for kc in range(KC):
                    nc.tensor.matmul(
                        qp[:, 0:w],
                        wsrc[:, kc, :],
                        srcs[kc][:, c0:c0 + w],
                        start=(kc == 0), stop=(kc == KC - 1),
                    )
                if dve_only:
                    nc.vector.tensor_copy(out=dst[:, c0:c0 + w],
                                          in_=qp[:, 0:w])
                else:
                    copy_alt(dst[:, c0:c0 + w], qp[:, 0:w])

            for ci, spec in enumerate(pair_chunks(0)):
                if ci % 2 == 0:
                    emit_chunk(spec)
                else:
                    emit_chunk(spec, pool=vps, tag="qp1")

        # ---- phase 2: attention per head PAIR, q in four quarters.
        # The two heads of a pair live at SBUF partitions 0-63 / 64-127 of
        # qts/kts, so their K=64 S^T matmuls land on disjoint PE row groups
        # and execute CONCURRENTLY (measured 152ns vs 467ns per N=512 mm).
        # Both heads' scores for one (kb, quarter) go into one [128,2,512]
        # PSUM tile so a single 1024-wide exp covers them.
        num_pool = ctx.enter_context(tc.tile_pool(name="num", bufs=1))
        # stacked normalized attn^T: 2 heads per tile (cat rows)
        nums = [num_pool.tile([128, T], BF16, tag=f"nm{j}", name=f"nm{j}")
                for j in range(NPAIR)]
        NQTR = T // 512          # 4 q-quarters
        # pools that outlive phase 2 (pair-3's normalize/flush interleaves
        # with the phase-3 tail)
        avsb_pool = ctx.enter_context(tc.tile_pool(name="av_sb", bufs=4))
        r_pool = ctx.enter_context(tc.tile_pool(name="r_sb", bufs=4))
        at_pool = ctx.enter_context(tc.tile_pool(name="at_sb", bufs=2))
        tps = ctx.enter_context(tc.tile_pool(name="tps", bufs=1, space="PSUM"))

        # deferred fine-grained PE work (normalize / flush), popped a
        # little per kb step so it never lumps into an ACT bubble
        deferred = []

        def pop_deferred(n, force=False):
            for _ in range(n):
                if deferred and (force or len(deferred) > 2):
                    deferred.pop(0)()

        def norm_one(av_sb, ap_tile, r0, tb, i):
            t1 = tps.tile([128, 128], F32, tag="tp", name="t1")
            nc.tensor.transpose(
                t1[:, 0:D_HEAD + 1],
                av_sb[:, i * 128:(i + 1) * 128],
                identity[0:D_HEAD + 1, 0:D_HEAD + 1],
            )
            r_sb = r_pool.tile([128, 1], F32, tag="r", name="r_sb")
            nc.vector.reciprocal(out=r_sb, in_=t1[:, D_HEAD:D_HEAD + 1])
            nc.vector.tensor_scalar_mul(
                ap_tile[:, tb, r0:r0 + 64], t1[:, 0:D_HEAD], r_sb)

        def flush_one(ap_tile, j, tb):
            t2 = tps.tile([128, 128], F32, tag="tp", name="t2")
            nc.tensor.transpose(t2, ap_tile[:, tb, :], identity)
            nc.vector.tensor_copy(
                out=nums[j][:, tb * 128:(tb + 1) * 128], in_=t2)

        # projection work of upcoming pairs, flattened to single-mm ops
        # popped 2 per kb step as PE filler
        chunk_ops = {}   # pair j -> list of closures

        def queue_chunk_ops(j):
            ops = []
            for spec in pair_chunks(j):
                kind, jj, c0, w = spec
                cell = {}
                for kc in range(KC):
                    def mm(kc=kc, kind=kind, jj=jj, c0=c0, w=w, cell=cell):
                        if kc == 0:
                            cell["qp"] = qkps.tile([128, 512], F32,
                                                   tag="qp", name="qp")
                        srcs = xqs if kind == "q" else xks
                        wsrc = wqs[jj] if kind == "q" else wks[jj]
                        nc.tensor.matmul(
                            cell["qp"][:, 0:w],
                            wsrc[:, kc, :],
                            srcs[kc][:, c0:c0 + w],
                            start=(kc == 0), stop=(kc == KC - 1),
                        )
                    ops.append(mm)

                def cp(kind=kind, jj=jj, c0=c0, w=w, cell=cell):
                    dst = qts[jj] if kind == "q" else kts[jj]
                    nc.vector.tensor_copy(out=dst[:, c0:c0 + w],
                                          in_=cell["qp"][:, 0:w])
                ops.append(cp)
            chunk_ops[j] = ops

        def pop_chunks(n):
            for _ in range(n):
                for j in sorted(chunk_ops):
                    if chunk_ops[j]:
                        chunk_ops[j].pop(0)()
                        break

        ap_last = [None]   # pair-3's ap tile, flushed during the tail
        with tc.tile_pool(name="p_sb", bufs=5) as p_pool, \
             tc.tile_pool(name="stps", bufs=2, space="PSUM") as stps, \
             tc.tile_pool(name="avps", bufs=1, space="PSUM") as avps:

            pair_state = {}   # pair -> {"ap":, "avp_q":}
            p_tiles = {}      # (pair, qtr, kb) -> p2

            def enter_pair(pair):
                for op in chunk_ops.pop(pair, []):   # safety: leftovers
                    op()
                if pair + 1 < NPAIR:
                    queue_chunk_ops(pair + 1)
                pair_state[pair] = {
                    "ap": at_pool.tile([128, NTB, 128], F32,
                                       tag="ap", name="ap"),
                    "avp_q": {},
                }

            def emit_st_exp(pair, qtr, kb):
                if qtr == 0 and kb == 0:
                    enter_pair(pair)
                q0 = qtr * 512
                stp2 = stps.tile([128, 2, 512], F32, tag="stp", name="stp2")
                nc.tensor.matmul(
                    stp2[:, 0, :],
                    kts[pair][0:64, kb * 128:(kb + 1) * 128],
                    qts[pair][0:64, q0:q0 + 512],
                    start=True, stop=True,
                )
                nc.tensor.matmul(
                    stp2[:, 1, :],
                    kts[pair][64:128, kb * 128:(kb + 1) * 128],
                    qts[pair][64:128, q0:q0 + 512],
                    start=True, stop=True,
                )
                p2 = p_pool.tile([128, 2, 512], BF16, tag="p", name="p2")
                nc.scalar.activation(
                    out=p2, in_=stp2,
                    func=mybir.ActivationFunctionType.Exp,
                    bias=maskb[:, kb:kb + 1], scale=0.125,
                )
                p_tiles[(pair, qtr, kb)] = p2

            def emit_av(pair, qtr, kb):
                st = pair_state[pair]
                ap_tile = st["ap"]
                avp_q = st["avp_q"]
                h0, h1 = 2 * pair, 2 * pair + 1
                if kb == 0:
                    avp_q[qtr] = (
                        avps.tile([D_HEAD + 1, 512], F32, tag="av0",
                                  name="avp0"),
                        avps.tile([D_HEAD + 1, 512], F32, tag="av1",
                                  name="avp1"),
                    )
                avp0, avp1 = avp_q[qtr]
                p2 = p_tiles.pop((pair, qtr, kb))
                nc.tensor.matmul(
                    avp0, vaugs[kb][:, h0, :], p2[:, 0, :],
                    start=(kb == 0), stop=(kb == NKB - 1),
                )
                nc.tensor.matmul(
                    avp1, vaugs[kb][:, h1, :], p2[:, 1, :],
                    start=(kb == 0), stop=(kb == NKB - 1),
                )
                if kb == NKB - 1:
                    # drain the accumulators; queue per-token-block
                    # normalize work (tb-ordered so the tail can consume)
                    av_sbs = []
                    for r0, avp in ((0, avp0), (64, avp1)):
                        av_sb = avsb_pool.tile(
                            [D_HEAD + 1, 512], F32, tag="avsb",
                            name="av_sb")
                        nc.vector.tensor_copy(out=av_sb, in_=avp)
                        av_sbs.append((r0, av_sb))
                    for i in range(4):
                        for r0, av_sb in av_sbs:
                            deferred.append(
                                lambda av_sb=av_sb, r0=r0, i=i,
                                tb=qtr * 4 + i, ap_tile=ap_tile:
                                norm_one(av_sb, ap_tile, r0, tb, i))
                    del avp_q[qtr]
                    if qtr == NQTR - 1:
                        if pair + 1 < NPAIR:
                            for tb in range(NTB):
                                deferred.append(
                                    lambda ap_tile=ap_tile, j=pair, tb=tb:
                                    flush_one(ap_tile, j, tb))
                        else:
                            ap_last[0] = ap_tile

            # ONE continuous kb stream across all pairs and quarters: the
            # ST->exp->AV lag spans every boundary so ACT never drains
            all_steps = [(p, q, k) for p in range(NPAIR)
                         for q in range(NQTR) for k in range(NKB)]
            for i, s in enumerate(all_steps):
                emit_st_exp(*s)
                if i >= LAG:
                    emit_av(*all_steps[i - LAG])
                pop_chunks(2)
                pop_deferred(2)
            for i in range(len(all_steps) - LAG, len(all_steps)):
                emit_av(*all_steps[i])
                pop_chunks(2)
                pop_deferred(2)
            # remaining deferred = pair-3's norms (tb-ordered); they and the
            # pair-3 flush interleave with the tail below

        # ---- phase 3: tail matmul  y[tok, out] = attn_cat @ wtailT,
        # pipelined per token block with pair-3's normalize/flush
        with tc.tile_pool(name="wt", bufs=1) as wt_pool, \
             tc.tile_pool(name="y_sb", bufs=3) as y_pool, \
             tc.tile_pool(name="yps", bufs=2, space="PSUM") as yps:

            def warm_keeper3():
                dmy3 = tps.tile([128, 128], F32, tag="tp", name="dmy3")
                nc.tensor.matmul(dmy3, identity, identity, start=True,
                                 stop=True)
            wts = [wt_pool.tile([128, D_MODEL], BF16, tag=f"wt{c}", name=f"wt{c}")
                   for c in range(CAT // 128)]
            for c in range(CAT // 128):
                nc.sync.dma_start(out=wts[c], in_=wtailT[c * 128:(c + 1) * 128, :])
            pop_deferred(4, force=True)   # prime norms for tb 0-1
            for tb in range(NTB):
                flush_one(ap_last[0], NPAIR - 1, tb)
                warm_keeper3()
                yp = yps.tile([128, D_MODEL], F32, tag="yp")
                for n in range(D_MODEL // 512):
                    for c in range(CAT // 128):
                        nc.tensor.matmul(
                            yp[:, n * 512:(n + 1) * 512],
                            nums[c][:, tb * 128:(tb + 1) * 128],
                            wts[c][:, n * 512:(n + 1) * 512],
                            start=(c == 0), stop=(c == CAT // 128 - 1),
                        )
                pop_deferred(2, force=True)   # norms for tb+2
                y_sb = y_pool.tile([128, D_MODEL], F32, tag="ys")
                copy_alt(y_sb, yp)
                nc.sync.dma_start(out=y[tb * 128:(tb + 1) * 128, :], in_=y_sb)

    if split_waits:
        split_excess_waits(nc)
    return nc


_NC_CACHE = {}


def _get_nc(kpad):
    if kpad not in _NC_CACHE:
        _NC_CACHE[kpad] = build_nc(kpad)
    return _NC_CACHE[kpad]


def _plan(x, mask, w_qkv, w_tail):
    """Compute KPAD from the mask and shard full inputs into 8 core maps."""
    bf = mybir.dt.np(BF16)
    x = np.asarray(x, dtype=np.float32)
    mask = np.asarray(mask, dtype=np.int32)
    w_qkv = np.asarray(w_qkv, dtype=np.float32)
    w_tail = np.asarray(w_tail, dtype=np.float32)

    idxs = [np.flatnonzero(mask[b]) for b in range(BN)]
    nk_max = max(len(i) for i in idxs)
    kpad = max(128, -(-nk_max // 128) * 128)

    # per-batch compacted k/v-side inputs
    xTs, xkTs, maskfs = [], [], []
    for b in range(BN):
        idx = idxs[b]
        xkb = np.zeros((kpad, D_MODEL), dtype=np.float32)
        xkb[:len(idx)] = x[b][idx]
        mf = np.full(kpad, -8e9, dtype=np.float32)
        mf[:len(idx)] = 0.0
        xTs.append(np.ascontiguousarray(x[b].T).astype(bf))
        xkTs.append(np.ascontiguousarray(xkb.T).astype(bf))
        maskfs.append(mf)

    w3 = w_qkv.reshape(N_HEAD, 3, D_HEAD, D_MODEL)  # [head, qkv, d, dmodel]
    in_maps = []
    for c in range(8):
        b, hg = divmod(c, 2)
        H = range(hg * HPC, (hg + 1) * HPC)
        wq = np.concatenate([w3[h, 0] for h in H], axis=0)  # [512, 1024]
        wk = np.concatenate([w3[h, 1] for h in H], axis=0)
        wv = np.concatenate([w3[h, 2] for h in H], axis=0)
        wt = w_tail[:, hg * CAT:(hg + 1) * CAT]  # [1024, 512]
        in_maps.append({
            "ident": np.eye(128, dtype=np.float32),
            "ones8": np.ones((128, HPC), dtype=bf),
            "xT": xTs[b],
            "xkT": xkTs[b],
            "maskf": maskfs[b],
            "wqT": np.ascontiguousarray(wq.T).astype(bf),
            "wkT": np.ascontiguousarray(wk.T).astype(bf),
            "wvT": np.ascontiguousarray(wv.T).astype(bf),
            "wtailT": np.ascontiguousarray(wt.T).astype(bf),
        })
    return kpad, in_maps


def kernel(x, mask, w_qkv, w_tail, b_tail):
    kpad, in_maps = _plan(x, mask, w_qkv, w_tail)
    nc = _get_nc(kpad)
    last_err = None
    for _attempt in range(3):
        try:
            res = run_bass_kernel_spmd(nc, in_maps, list(range(8))).results
            break
        except Exception as e:  # transient device/runtime errors: retry
            last_err = e
            _time.sleep(3.0)
    else:
        raise last_err
    out = np.empty((BN, T, D_MODEL), dtype=np.float32)
    b_tail = np.asarray(b_tail, dtype=np.float32)
    for b in range(BN):
        out[b] = res[2 * b]["y"] + res[2 * b + 1]["y"] + b_tail
    return out


# revision 40
# speedup vs baseline: 1.3179x; 1.0037x over previous
"""Multi-head attention Trainium2 kernel, 8-way sharded, mask-compacted keys.

Problem: x[4,2048,1024] -> qkv proj (w_qkv [3072,1024]) -> 16-head attention
with key-padding mask -> tail proj (w_tail [1024,1024]) + b_tail.

Sharding: 8 shards = 4 batches x 2 head-groups (8 heads each). Each core
computes, for its (batch b, head-group hg):
  - q projection of x[b] (all T tokens) for its 8 heads
  - k/v projections of the mask-COMPACTED tokens of x[b] (keys with mask=0
    contribute exp(-inf)=0 to softmax, so they are dropped host-side and
    the key axis padded to KPAD, a multiple of 128; pads get bias -8e9)
  - [T x KPAD] masked attention per head
  - partial tail matmul y_part = attn_cat @ w_tail[:, cat_slice].T
Host unshards: out[b] = y_part[2b] + y_part[2b+1] + b_tail.  No collectives.

Engine strategy (trace-driven):
  - phase 1 (projections) and phase 3 (tail) are PE-dense: bf16 operands
    (1 cyc/row, background weight loads), PSUM->SBUF copies alternate
    between DVE and ACT so neither serializes the PE.
  - phase 2 (attention) is paced by ACT exp ([128,1024] tiles); PE operands
    stay float32r: the serial 4-byte weight load pads PE occupancy to
    ~match ACT, keeping the HAM clock at 8/8 (bf16 here made the PE idle
    23% per kb and the HAM halved the clock for ~180us).
"""

import time as _time

import numpy as np
from contextlib import ExitStack

import concourse.bass as bass
import concourse.mybir as mybir
import concourse.tile as tile
from concourse.bass_utils import run_bass_kernel_spmd

# ---------------------------------------------------------------------------
# walrus in this env accepts at most 2 sync waits per instruction; Tile's
# scheduler emits up to 10. Post-pass: peel excess waits onto same-engine
# NoOps inserted immediately before the offending instruction (same engine
# stream position => identical synchronization semantics).
MAX_WAITS = 1


def split_excess_waits(nc):
    for fn in nc.m.functions:
        for bb in fn.blocks:
            insts = list(bb.instructions)
            out = []
            changed = False
            for inst in insts:
                si = inst.sync_info
                waits = list(si.on_wait) if si is not None else []
                if len(waits) > MAX_WAITS:
                    extra = waits[:-MAX_WAITS]
                    for ci in range(0, len(extra), MAX_WAITS):
                        chunk = extra[ci:ci + MAX_WAITS]
                        nop = mybir.InstNoOp(
                            name=f"{inst.name}-ws{ci}", ins=[], outs=[])
                        nop.engine = inst.engine
                        nop.sync_info = mybir.SyncInfo(
                            on_wait=chunk, on_update=[])
                        out.append(nop)
                    inst.sync_info = mybir.SyncInfo(
                        on_wait=waits[-MAX_WAITS:],
                        on_update=list(si.on_update))
                    changed = True
                out.append(inst)
            if changed:
                bb.instructions = out
# ---------------------------------------------------------------------------

D_MODEL = 1024
N_HEAD = 16
D_HEAD = 64
BN, T = 4, 2048
HPC = 8                      # heads per core
NPAIR = HPC // 2             # head pairs (q/k tiles hold 2 heads)
CAT = HPC * D_HEAD           # 512 per-core tail contraction
NTB = T // 128               # 16 query-token blocks
QH = T // 2                  # 1024, q processed in two halves
KC = D_MODEL // 128          # 8 contraction chunks
F32 = mybir.dt.float32
F32R = mybir.dt.float32r
BF16 = mybir.dt.bfloat16
I32 = mybir.dt.int32


def build_nc(kpad, split_waits=True):
    assert kpad % 128 == 0 and 128 <= kpad <= T
    NKB = kpad // 128        # key blocks
    LAG = min(4, NKB - 1) if NKB > 1 else 0
    # k-projection chunks: (start, width), width 512 or the tail remainder
    KCH = [(c * 512, min(512, kpad - c * 512))
           for c in range((kpad + 511) // 512)]

    nc = bass.Bass()
    xT = nc.declare_dram_parameter("xT", [D_MODEL, T], BF16, isOutput=False)
    xkT = nc.declare_dram_parameter("xkT", [D_MODEL, kpad], BF16, isOutput=False)
    wqT = nc.declare_dram_parameter("wqT", [D_MODEL, CAT], BF16, isOutput=False)
    wkT = nc.declare_dram_parameter("wkT", [D_MODEL, CAT], BF16, isOutput=False)
    wvT = nc.declare_dram_parameter("wvT", [D_MODEL, CAT], BF16, isOutput=False)
    wtailT = nc.declare_dram_parameter("wtailT", [CAT, D_MODEL], BF16, isOutput=False)
    maskf = nc.declare_dram_parameter("maskf", [kpad], F32, isOutput=False)
    ident = nc.declare_dram_parameter("ident", [128, 128], F32, isOutput=False)
    ones8 = nc.declare_dram_parameter("ones8", [128, HPC], BF16, isOutput=False)
    y = nc.declare_dram_parameter("y", [T, D_MODEL], F32, isOutput=True)

    with ExitStack() as ctx:
        tc = ctx.enter_context(tile.TileContext(nc))

        # ---- long-lived pools (entered first so short-lived ones stack on top)
        const = ctx.enter_context(tc.tile_pool(name="const", bufs=1))
        qk_pool = ctx.enter_context(tc.tile_pool(name="qk", bufs=1))
        vaug_pool = ctx.enter_context(tc.tile_pool(name="vaug", bufs=1))

        identity = const.tile([128, 128], F32)
        nc.sync.dma_start(out=identity, in_=ident[:, :])

        # per-key-block additive exp bias: 0 for kept keys, -8e9 for pads
        maskb = const.tile([128, NKB], F32)
        nc.sync.dma_start(out=maskb, in_=maskf.rearrange("(j p) -> p j", p=128))

        # persistent intermeds
        # q/k of 2 heads per tile: rows [h0 d64 | h1 d64]
        qts = [qk_pool.tile([128, T], BF16, tag=f"qt{j}", name=f"qt{j}")
               for j in range(NPAIR)]
        kts = [qk_pool.tile([128, kpad], BF16, tag=f"kt{j}", name=f"kt{j}")
               for j in range(NPAIR)]
        # V augmented with ones column: [key-block][128, head, 65]
        vaugs = [vaug_pool.tile([128, HPC, D_HEAD + 1], BF16,
                                tag=f"va{t}", name=f"va{t}")
                 for t in range(NKB)]

        # alternate PSUM->SBUF copies between DVE and ACT so neither engine
        # serializes the PE in the projection phase
        _cp = [0]

        def copy_alt(out, in_):
            if _cp[0] % 2 == 0:
                nc.vector.tensor_copy(out=out, in_=in_)
            else:
                nc.scalar.activation(
                    out=out, in_=in_, func=mybir.ActivationFunctionType.Copy)
            _cp[0] += 1

        # x + projection weights stay resident for the whole kernel so the
        # q/k projections of pairs 1-3 can interleave into phase 2 (they are
        # the PE's filler work while ACT computes exp).
        xw_pool = ctx.enter_context(tc.tile_pool(name="xw", bufs=1))
        qkps = ctx.enter_context(tc.tile_pool(name="qkps", bufs=1, space="PSUM"))

        # ---- phase 1: V projection + pair-0 q/k projection
        with tc.tile_pool(name="vps", bufs=1, space="PSUM") as vps:
            # spread input DMAs over the three DMA-issuing engines (SP,
            # ACT, GPSIMD) so the first V-proj matmul only waits for
            # xk[0]+wv[0], not a 20us serial DMA chain
            xks = [xw_pool.tile([128, kpad], BF16, tag=f"xk{kc}", name=f"xk{kc}")
                   for kc in range(KC)]
            wvs = [xw_pool.tile([128, CAT], BF16, tag=f"wv{kc}", name=f"wv{kc}")
                   for kc in range(KC)]
            for kc in range(KC):
                nc.sync.dma_start(out=xks[kc],
                                  in_=xkT[kc * 128:(kc + 1) * 128, :])
                nc.sync.dma_start(out=wvs[kc],
                                  in_=wvT[kc * 128:(kc + 1) * 128, :])
            xqs = [xw_pool.tile([128, T], BF16, tag=f"xq{kc}", name=f"xq{kc}")
                   for kc in range(KC)]
            for kc in range(KC):
                nc.scalar.dma_start(out=xqs[kc],
                                    in_=xT[kc * 128:(kc + 1) * 128, :])
            wqs = [xw_pool.tile([128, KC, 128], BF16, tag=f"wq{j}", name=f"wq{j}")
                   for j in range(NPAIR)]
            wks = [xw_pool.tile([128, KC, 128], BF16, tag=f"wk{j}", name=f"wk{j}")
                   for j in range(NPAIR)]
            for j in range(NPAIR):
                nc.scalar.dma_start(
                    out=wqs[j],
                    in_=wqT.rearrange("(kc p) c -> p kc c", p=128)[
                        :, :, j * 128:(j + 1) * 128])
                nc.scalar.dma_start(
                    out=wks[j],
                    in_=wkT.rearrange("(kc p) c -> p kc c", p=128)[
                        :, :, j * 128:(j + 1) * 128])

            # V projection over compacted keys: V[key, cat] = xk @ Wv^T.
            # kc-outer over groups of 7 live PSUM banks so compute starts
            # as soon as the first xk/wv tile pair lands.
            for g0 in range(0, NKB, 6):
                tbs = range(g0, min(g0 + 6, NKB))
                vp7 = {tb: vps.tile([128, CAT], F32, tag=f"vp{tb - g0}",
                                    name=f"vp{tb}") for tb in tbs}
                for kc in range(KC):
                    for tb in tbs:
                        nc.tensor.matmul(
                            vp7[tb],
                            xks[kc][:, tb * 128:(tb + 1) * 128],
                            wvs[kc],
                            start=(kc == 0), stop=(kc == KC - 1),
                        )
                for tb in tbs:
                    va = vaugs[tb]
                    nc.sync.dma_start(
                        out=va[:, :, D_HEAD:D_HEAD + 1], in_=ones8[:, :])
                    copy_alt(va[:, :, 0:D_HEAD],
                             vp7[tb].rearrange("p (h d) -> p h d", h=HPC))

            # Q projection (full T) and K projection (kpad), per head pair:
            # out rows = [q(2j) 64 | q(2j+1) 64] so one [128, chunk] copy
            # moves both heads at once.  Only pair 0 runs in phase 1; pairs
            # 1-3 are emitted chunk-by-chunk inside phase 2.
            def pair_chunks(j):
                return ([("q", j, c * 512, 512) for c in range(T // 512)]
                        + [("k", j, c0, w) for (c0, w) in KCH])

            def emit_chunk(spec, dve_only=False, pool=None, tag="qp"):
                kind, j, c0, w = spec
                qp = (pool or qkps).tile([128, 512], F32, tag=tag, name="qp")
                srcs = xqs if kind == "q" else xks
                wsrc = wqs[j] if kind == "q" else wks[j]
                dst = qts[j] if kind == "q" else kts[j]
                for kc in range(KC):
                    nc.tensor.matmul(
                        qp[:, 0:w],
                        wsrc[:, kc, :],
                        srcs[kc][:, c0:c0 + w],
                        start=(kc == 0), stop=(kc == KC - 1),
                    )
                if dve_only:
                    nc.vector.tensor_copy(out=dst[:, c0:c0 + w],
                                          in_=qp[:, 0:w])
                else:
                    copy_alt(dst[:, c0:c0 + w], qp[:, 0:w])

            for ci, spec in enumerate(pair_chunks(0)):
                if ci % 2 == 0:
                    emit_chunk(spec)
                else:
                    emit_chunk(spec, pool=vps, tag="qp1")

        # ---- phase 2: attention per head PAIR, q in four quarters.
        # The two heads of a pair live at SBUF partitions 0-63 / 64-127 of
        # qts/kts, so their K=64 S^T matmuls land on disjoint PE row groups
        # and execute CONCURRENTLY (measured 152ns vs 467ns per N=512 mm).
        # Both heads' scores for one (kb, quarter) go into one [128,2,512]
        # PSUM tile so a single 1024-wide exp covers them.
        num_pool = ctx.enter_context(tc.tile_pool(name="num", bufs=1))
        # stacked normalized attn^T: 2 heads per tile (cat rows)
        nums = [num_pool.tile([128, T], BF16, tag=f"nm{j}", name=f"nm{j}")
                for j in range(NPAIR)]
        NQTR = T // 512          # 4 q-quarters
        # pools that outlive phase 2 (pair-3's normalize/flush interleaves
        # with the phase-3 tail)
        avsb_pool = ctx.enter_context(tc.tile_pool(name="av_sb", bufs=4))
        r_pool = ctx.enter_context(tc.tile_pool(name="r_sb", bufs=4))
        at_pool = ctx.enter_context(tc.tile_pool(name="at_sb", bufs=2))
        tps = ctx.enter_context(tc.tile_pool(name="tps", bufs=1, space="PSUM"))

        # deferred fine-grained PE work (normalize / flush), popped a
        # little per kb step so it never lumps into an ACT bubble
        deferred = []

        def pop_deferred(n, force=False):
            for _ in range(n):
                if deferred and (force or len(deferred) > 2):
                    deferred.pop(0)()

        def norm_one(av_sb, ap_tile, r0, tb, i):
            t1 = tps.tile([128, 128], F32, tag="tp", name="t1")
            nc.tensor.transpose(
                t1[:, 0:D_HEAD + 1],
                av_sb[:, i * 128:(i + 1) * 128],
                identity[0:D_HEAD + 1, 0:D_HEAD + 1],
            )
            r_sb = r_pool.tile([128, 1], F32, tag="r", name="r_sb")
            nc.vector.reciprocal(out=r_sb, in_=t1[:, D_HEAD:D_HEAD + 1])
            nc.vector.tensor_scalar_mul(
                ap_tile[:, tb, r0:r0 + 64], t1[:, 0:D_HEAD], r_sb)

        def flush_one(ap_tile, j, tb):
            t2 = tps.tile([128, 128], F32, tag="tp", name="t2")
            nc.tensor.transpose(t2, ap_tile[:, tb, :], identity)
            nc.vector.tensor_copy(
                out=nums[j][:, tb * 128:(tb + 1) * 128], in_=t2)

        # projection work of upcoming pairs, flattened to single-mm ops
        # popped 2 per kb step as PE filler
        chunk_ops = {}   # pair j -> list of closures

        def queue_chunk_ops(j):
            ops = []
            for spec in pair_chunks(j):
                kind, jj, c0, w = spec
                cell = {}
                for kc in range(KC):
                    def mm(kc=kc, kind=kind, jj=jj, c0=c0, w=w, cell=cell):
                        if kc == 0:
                            cell["qp"] = qkps.tile([128, 512], F32,
                                                   tag="qp", name="qp")
                        srcs = xqs if kind == "q" else xks
                        wsrc = wqs[jj] if kind == "q" else wks[jj]
                        nc.tensor.matmul(
                            cell["qp"][:, 0:w],
                            wsrc[:, kc, :],
                            srcs[kc][:, c0:c0 + w],
                            start=(kc == 0), stop=(kc == KC - 1),
                        )
                    ops.append(mm)

                def cp(kind=kind, jj=jj, c0=c0, w=w, cell=cell):
                    dst = qts[jj] if kind == "q" else kts[jj]
                    nc.vector.tensor_copy(out=dst[:, c0:c0 + w],
                                          in_=cell["qp"][:, 0:w])
                ops.append(cp)
            chunk_ops[j] = ops

        def pop_chunks(n):
            for _ in range(n):
                for j in sorted(chunk_ops):
                    if chunk_ops[j]:
                        chunk_ops[j].pop(0)()
                        break

        ap_last = [None]   # pair-3's ap tile, flushed during the tail
        with tc.tile_pool(name="p_sb", bufs=5) as p_pool, \
             tc.tile_pool(name="stps", bufs=2, space="PSUM") as stps, \
             tc.tile_pool(name="avps", bufs=1, space="PSUM") as avps:

            pair_state = {}   # pair -> {"ap":, "avp_q":}
            p_tiles = {}      # (pair, qtr, kb) -> p2

            def enter_pair(pair):
                for op in chunk_ops.pop(pair, []):   # safety: leftovers
                    op()
                if pair + 1 < NPAIR:
                    queue_chunk_ops(pair + 1)
                pair_state[pair] = {
                    "ap": at_pool.tile([128, NTB, 128], F32,
                                       tag="ap", name="ap"),
                    "avp_q": {},
                }

            def emit_st_exp(pair, qtr, kb):
                if qtr == 0 and kb == 0:
                    enter_pair(pair)
                q0 = qtr * 512
                stp2 = stps.tile([128, 2, 512], F32, tag="stp", name="stp2")
                nc.tensor.matmul(
                    stp2[:, 0, :],
                    kts[pair][0:64, kb * 128:(kb + 1) * 128],
                    qts[pair][0:64, q0:q0 + 512],
                    start=True, stop=True,
                )
                nc.tensor.matmul(
                    stp2[:, 1, :],
                    kts[pair][64:128, kb * 128:(kb + 1) * 128],
                    qts[pair][64:128, q0:q0 + 512],
                    start=True, stop=True,
                )
                p2 = p_pool.tile([128, 2, 512], BF16, tag="p", name="p2")
                nc.scalar.activation(
                    out=p2, in_=stp2,
                    func=mybir.ActivationFunctionType.Exp,
                    bias=maskb[:, kb:kb + 1], scale=0.125,
                )
                p_tiles[(pair, qtr, kb)] = p2

            def emit_av(pair, qtr, kb):
                st = pair_state[pair]
                ap_tile = st["ap"]
                avp_q = st["avp_q"]
                h0, h1 = 2 * pair, 2 * pair + 1
                if kb == 0:
                    avp_q[qtr] = (
                        avps.tile([D_HEAD + 1, 512], F32, tag="av0",
                                  name="avp0"),
                        avps.tile([D_HEAD + 1, 512], F32, tag="av1",
                                  name="avp1"),
                    )
                avp0, avp1 = avp_q[qtr]
                p2 = p_tiles.pop((pair, qtr, kb))
                nc.tensor.matmul(
                    avp0, vaugs[kb][:, h0, :], p2[:, 0, :],
                    start=(kb == 0), stop=(kb == NKB - 1),
                )
                nc.tensor.matmul(
                    avp1, vaugs[kb][:, h1, :], p2[:, 1, :],
                    start=(kb == 0), stop=(kb == NKB - 1),
                )
                if kb == NKB - 1:
                    # drain the accumulators; queue per-token-block
                    # normalize work (tb-ordered so the tail can consume)
                    av_sbs = []
                    for r0, avp in ((0, avp0), (64, avp1)):
                        av_sb = avsb_pool.tile(
                            [D_HEAD + 1, 512], F32, tag="avsb",
                            name="av_sb")
                        nc.vector.tensor_copy(out=av_sb, in_=avp)
                        av_sbs.append((r0, av_sb))
                    for i in range(4):
                        for r0, av_sb in av_sbs:
                            deferred.append(
                                lambda av_sb=av_sb, r0=r0, i=i,
                                tb=qtr * 4 + i, ap_tile=ap_tile:
                                norm_one(av_sb, ap_tile, r0, tb, i))
                    del avp_q[qtr]
                    if qtr == NQTR - 1:
                        if pair + 1 < NPAIR:
                            for tb in range(NTB):
                                deferred.append(
                                    lambda ap_tile=ap_tile, j=pair, tb=tb:
                                    flush_one(ap_tile, j, tb))
                        else:
                            ap_last[0] = ap_tile

            # ONE continuous kb stream across all pairs and quarters: the
            # ST->exp->AV lag spans every boundary so ACT never drains
            all_steps = [(p, q, k) for p in range(NPAIR)
                         for q in range(NQTR) for k in range(NKB)]
            for i, s in enumerate(all_steps):
                emit_st_exp(*s)
                if i >= LAG:
                    emit_av(*all_steps[i - LAG])
                pop_chunks(2)
                pop_deferred(2)
            for i in range(len(all_steps) - LAG, len(all_steps)):
                emit_av(*all_steps[i])
                pop_chunks(2)
                pop_deferred(2)
            # remaining deferred = pair-3's norms (tb-ordered); they and the
            # pair-3 flush interleave with the tail below

        # ---- phase 3: tail matmul  y[tok, out] = attn_cat @ wtailT,
        # pipelined per token block with pair-3's normalize/flush
        with tc.tile_pool(name="wt", bufs=1) as wt_pool, \
             tc.tile_pool(name="y_sb", bufs=3) as y_pool, \
             tc.tile_pool(name="yps", bufs=2, space="PSUM") as yps:

            def warm_keeper3():
                dmy3 = tps.tile([128, 128], F32, tag="tp", name="dmy3")
                nc.tensor.matmul(dmy3, identity, identity, start=True,
                                 stop=True)
            wts = [wt_pool.tile([128, D_MODEL], BF16, tag=f"wt{c}", name=f"wt{c}")
                   for c in range(CAT // 128)]
            for c in range(CAT // 128):
                nc.sync.dma_start(out=wts[c], in_=wtailT[c * 128:(c + 1) * 128, :])
            pop_deferred(4, force=True)   # prime norms for tb 0-1
            for tb in range(NTB):
                flush_one(ap_last[0], NPAIR - 1, tb)
                warm_keeper3()
                yp = yps.tile([128, D_MODEL], F32, tag="yp")
                for n in range(D_MODEL // 512):
                    for c in range(CAT // 128):
                        nc.tensor.matmul(
                            yp[:, n * 512:(n + 1) * 512],
                            nums[c][:, tb * 128:(tb + 1) * 128],
                            wts[c][:, n * 512:(n + 1) * 512],
                            start=(c == 0), stop=(c == CAT // 128 - 1),
                        )
                pop_deferred(2, force=True)   # norms for tb+2
                y_sb = y_pool.tile([128, D_MODEL], F32, tag="ys")
                copy_alt(y_sb, yp)
                nc.sync.dma_start(out=y[tb * 128:(tb + 1) * 128, :], in_=y_sb)

    if split_waits:
        split_excess_waits(nc)
    return nc


_NC_CACHE = {}


def _get_nc(kpad):
    if kpad not in _NC_CACHE:
        _NC_CACHE[kpad] = build_nc(kpad)
    return _NC_CACHE[kpad]


def _plan(x, mask, w_qkv, w_tail):
    """Compute KPAD from the mask and shard full inputs into 8 core maps."""
    bf = mybir.dt.np(BF16)
    x = np.asarray(x, dtype=np.float32)
    mask = np.asarray(mask, dtype=np.int32)
    w_qkv = np.asarray(w_qkv, dtype=np.float32)
    w_tail = np.asarray(w_tail, dtype=np.float32)

    idxs = [np.flatnonzero(mask[b]) for b in range(BN)]
    nk_max = max(len(i) for i in idxs)
    kpad = max(128, -(-nk_max // 128) * 128)

    # per-batch compacted k/v-side inputs
    xTs, xkTs, maskfs = [], [], []
    for b in range(BN):
        idx = idxs[b]
        xkb = np.zeros((kpad, D_MODEL), dtype=np.float32)
        xkb[:len(idx)] = x[b][idx]
        mf = np.full(kpad, -8e9, dtype=np.float32)
        mf[:len(idx)] = 0.0
        xTs.append(np.ascontiguousarray(x[b].T).astype(bf))
        xkTs.append(np.ascontiguousarray(xkb.T).astype(bf))
        maskfs.append(mf)

    w3 = w_qkv.reshape(N_HEAD, 3, D_HEAD, D_MODEL)  # [head, qkv, d, dmodel]
    in_maps = []
    for c in range(8):
        b, hg = divmod(c, 2)
        H = range(hg * HPC, (hg + 1) * HPC)
        wq = np.concatenate([w3[h, 0] for h in H], axis=0)  # [512, 1024]
        wk = np.concatenate([w3[h, 1] for h in H], axis=0)
        wv = np.concatenate([w3[h, 2] for h in H], axis=0)
        wt = w_tail[:, hg * CAT:(hg + 1) * CAT]  # [1024, 512]
        in_maps.append({
            "ident": np.eye(128, dtype=np.float32),
            "ones8": np.ones((128, HPC), dtype=bf),
            "xT": xTs[b],
            "xkT": xkTs[b],
            "maskf": maskfs[b],
            "wqT": np.ascontiguousarray(wq.T).astype(bf),
            "wkT": np.ascontiguousarray(wk.T).astype(bf),
            "wvT": np.ascontiguousarray(wv.T).astype(bf),
            "wtailT": np.ascontiguousarray(wt.T).astype(bf),
        })
    return kpad, in_maps


def kernel(x, mask, w_qkv, w_tail, b_tail):
    kpad, in_maps = _plan(x, mask, w_qkv, w_tail)
    nc = _get_nc(kpad)
    last_err = None
    for _attempt in range(3):
        try:
            res = run_bass_kernel_spmd(nc, in_maps, list(range(8))).results
            break
        except Exception as e:  # transient device/runtime errors: retry
            last_err = e
            _time.sleep(3.0)
    else:
        raise last_err
    out = np.empty((BN, T, D_MODEL), dtype=np.float32)
    b_tail = np.asarray(b_tail, dtype=np.float32)
    for b in range(BN):
        out[b] = res[2 * b]["y"] + res[2 * b + 1]["y"] + b_tail
    return out


# revision 42
# speedup vs baseline: 1.3280x; 1.0076x over previous
"""Multi-head attention Trainium2 kernel, 8-way sharded, mask-compacted keys.

Problem: x[4,2048,1024] -> qkv proj (w_qkv [3072,1024]) -> 16-head attention
with key-padding mask -> tail proj (w_tail [1024,1024]) + b_tail.

Sharding: 8 shards = 4 batches x 2 head-groups (8 heads each). Each core
computes, for its (batch b, head-group hg):
  - q projection of x[b] (all T tokens) for its 8 heads
  - k/v projections of the mask-COMPACTED tokens of x[b] (keys with mask=0
    contribute exp(-inf)=0 to softmax, so they are dropped host-side and
    the key axis padded to KPAD, a multiple of 128; pads get bias -8e9)
  - [T x KPAD] masked attention per head
  - partial tail matmul y_part = attn_cat @ w_tail[:, cat_slice].T
Host unshards: out[b] = y_part[2b] + y_part[2b+1] + b_tail.  No collectives.

Engine strategy (trace-driven):
  - phase 1 (projections) and phase 3 (tail) are PE-dense: bf16 operands
    (1 cyc/row, background weight loads), PSUM->SBUF copies alternate
    between DVE and ACT so neither serializes the PE.
  - phase 2 (attention) is paced by ACT exp ([128,1024] tiles); PE operands
    stay float32r: the serial 4-byte weight load pads PE occupancy to
    ~match ACT, keeping the HAM clock at 8/8 (bf16 here made the PE idle
    23% per kb and the HAM halved the clock for ~180us).
"""

import time as _time

import numpy as np
from contextlib import ExitStack

import concourse.bass as bass
import concourse.mybir as mybir
import concourse.tile as tile
from concourse.bass_utils import run_bass_kernel_spmd

# ---------------------------------------------------------------------------
# walrus in this env accepts at most 2 sync waits per instruction; Tile's
# scheduler emits up to 10. Post-pass: peel excess waits onto same-engine
# NoOps inserted immediately before the offending instruction (same engine
# stream position => identical synchronization semantics).
MAX_WAITS = 1


def split_excess_waits(nc):
    for fn in nc.m.functions:
        for bb in fn.blocks:
            insts = list(bb.instructions)
            out = []
            changed = False
            for inst in insts:
                si = inst.sync_info
                waits = list(si.on_wait) if si is not None else []
                if len(waits) > MAX_WAITS:
                    extra = waits[:-MAX_WAITS]
                    for ci in range(0, len(extra), MAX_WAITS):
                        chunk = extra[ci:ci + MAX_WAITS]
                        nop = mybir.InstNoOp(
                            name=f"{inst.name}-ws{ci}", ins=[], outs=[])
                        nop.engine = inst.engine
                        nop.sync_info = mybir.SyncInfo(
                            on_wait=chunk, on_update=[])
                        out.append(nop)
                    inst.sync_info = mybir.SyncInfo(
                        on_wait=waits[-MAX_WAITS:],
                        on_update=list(si.on_update))
                    changed = True
                out.append(inst)
            if changed:
                bb.instructions = out
# ---------------------------------------------------------------------------

D_MODEL = 1024
N_HEAD = 16
D_HEAD = 64
BN, T = 4, 2048
HPC = 8                      # heads per core
NPAIR = HPC // 2             # head pairs (q/k tiles hold 2 heads)
CAT = HPC * D_HEAD           # 512 per-core tail contraction
NTB = T // 128               # 16 query-token blocks
QH = T // 2                  # 1024, q processed in two halves
KC = D_MODEL // 128          # 8 contraction chunks
F32 = mybir.dt.float32
F32R = mybir.dt.float32r
BF16 = mybir.dt.bfloat16
I32 = mybir.dt.int32


def build_nc(kpad, split_waits=True):
    assert kpad % 128 == 0 and 128 <= kpad <= T
    NKB = kpad // 128        # key blocks
    LAG = min(4, NKB - 1) if NKB > 1 else 0
    # k-projection chunks: (start, width), width 512 or the tail remainder
    KCH = [(c * 512, min(512, kpad - c * 512))
           for c in range((kpad + 511) // 512)]

    nc = bass.Bass()
    xT = nc.declare_dram_parameter("xT", [D_MODEL, T], BF16, isOutput=False)
    xkT = nc.declare_dram_parameter("xkT", [D_MODEL, kpad], BF16, isOutput=False)
    wqT = nc.declare_dram_parameter("wqT", [D_MODEL, CAT], BF16, isOutput=False)
    wkT = nc.declare_dram_parameter("wkT", [D_MODEL, CAT], BF16, isOutput=False)
    wvT = nc.declare_dram_parameter("wvT", [D_MODEL, CAT], BF16, isOutput=False)
    wtailT = nc.declare_dram_parameter("wtailT", [CAT, D_MODEL], BF16, isOutput=False)
    maskf = nc.declare_dram_parameter("maskf", [kpad], F32, isOutput=False)
    ident = nc.declare_dram_parameter("ident", [128, 128], F32, isOutput=False)
    identb = nc.declare_dram_parameter("identb", [128, 128], BF16, isOutput=False)
    ones8 = nc.declare_dram_parameter("ones8", [128, HPC], BF16, isOutput=False)
    y = nc.declare_dram_parameter("y", [T, D_MODEL], F32, isOutput=True)

    with ExitStack() as ctx:
        tc = ctx.enter_context(tile.TileContext(nc))

        # ---- long-lived pools (entered first so short-lived ones stack on top)
        const = ctx.enter_context(tc.tile_pool(name="const", bufs=1))
        qk_pool = ctx.enter_context(tc.tile_pool(name="qk", bufs=1))
        vaug_pool = ctx.enter_context(tc.tile_pool(name="vaug", bufs=1))

        identity = const.tile([128, 128], F32)
        nc.sync.dma_start(out=identity, in_=ident[:, :])
        identB = const.tile([128, 128], BF16)
        nc.sync.dma_start(out=identB, in_=identb[:, :])

        # per-key-block additive exp bias: 0 for kept keys, -8e9 for pads
        maskb = const.tile([128, NKB], F32)
        nc.sync.dma_start(out=maskb, in_=maskf.rearrange("(j p) -> p j", p=128))

        # persistent intermeds
        # q/k of 2 heads per tile: rows [h0 d64 | h1 d64]
        qts = [qk_pool.tile([128, T], BF16, tag=f"qt{j}", name=f"qt{j}")
               for j in range(NPAIR)]
        kts = [qk_pool.tile([128, kpad], BF16, tag=f"kt{j}", name=f"kt{j}")
               for j in range(NPAIR)]
        # V augmented with ones column: [key-block][128, head, 65]
        vaugs = [vaug_pool.tile([128, HPC, D_HEAD + 1], BF16,
                                tag=f"va{t}", name=f"va{t}")
                 for t in range(NKB)]

        # alternate PSUM->SBUF copies between DVE and ACT so neither engine
        # serializes the PE in the projection phase
        _cp = [0]

        def copy_alt(out, in_):
            if _cp[0] % 2 == 0:
                nc.vector.tensor_copy(out=out, in_=in_)
            else:
                nc.scalar.activation(
                    out=out, in_=in_, func=mybir.ActivationFunctionType.Copy)
            _cp[0] += 1

        # x + projection weights stay resident for the whole kernel so the
        # q/k projections of pairs 1-3 can interleave into phase 2 (they are
        # the PE's filler work while ACT computes exp).
        xw_pool = ctx.enter_context(tc.tile_pool(name="xw", bufs=1))
        qkps = ctx.enter_context(tc.tile_pool(name="qkps", bufs=1, space="PSUM"))

        # ---- phase 1: V projection + pair-0 q/k projection
        with tc.tile_pool(name="vps", bufs=1, space="PSUM") as vps:
            # spread input DMAs over the three DMA-issuing engines (SP,
            # ACT, GPSIMD) so the first V-proj matmul only waits for
            # xk[0]+wv[0], not a 20us serial DMA chain
            xks = [xw_pool.tile([128, kpad], BF16, tag=f"xk{kc}", name=f"xk{kc}")
                   for kc in range(KC)]
            wvs = [xw_pool.tile([128, CAT], BF16, tag=f"wv{kc}", name=f"wv{kc}")
                   for kc in range(KC)]
            for kc in range(KC):
                nc.sync.dma_start(out=xks[kc],
                                  in_=xkT[kc * 128:(kc + 1) * 128, :])
                nc.sync.dma_start(out=wvs[kc],
                                  in_=wvT[kc * 128:(kc + 1) * 128, :])
            xqs = [xw_pool.tile([128, T], BF16, tag=f"xq{kc}", name=f"xq{kc}")
                   for kc in range(KC)]
            for kc in range(KC):
                nc.scalar.dma_start(out=xqs[kc],
                                    in_=xT[kc * 128:(kc + 1) * 128, :])
            wqs = [xw_pool.tile([128, KC, 128], BF16, tag=f"wq{j}", name=f"wq{j}")
                   for j in range(NPAIR)]
            wks = [xw_pool.tile([128, KC, 128], BF16, tag=f"wk{j}", name=f"wk{j}")
                   for j in range(NPAIR)]
            for j in range(NPAIR):
                nc.scalar.dma_start(
                    out=wqs[j],
                    in_=wqT.rearrange("(kc p) c -> p kc c", p=128)[
                        :, :, j * 128:(j + 1) * 128])
                nc.scalar.dma_start(
                    out=wks[j],
                    in_=wkT.rearrange("(kc p) c -> p kc c", p=128)[
                        :, :, j * 128:(j + 1) * 128])

            # V projection over compacted keys: V[key, cat] = xk @ Wv^T.
            # kc-outer over groups of 7 live PSUM banks so compute starts
            # as soon as the first xk/wv tile pair lands.
            for g0 in range(0, NKB, 6):
                tbs = range(g0, min(g0 + 6, NKB))
                vp7 = {tb: vps.tile([128, CAT], F32, tag=f"vp{tb - g0}",
                                    name=f"vp{tb}") for tb in tbs}
                for kc in range(KC):
                    for tb in tbs:
                        nc.tensor.matmul(
                            vp7[tb],
                            xks[kc][:, tb * 128:(tb + 1) * 128],
                            wvs[kc],
                            start=(kc == 0), stop=(kc == KC - 1),
                        )
                for tb in tbs:
                    va = vaugs[tb]
                    nc.sync.dma_start(
                        out=va[:, :, D_HEAD:D_HEAD + 1], in_=ones8[:, :])
                    copy_alt(va[:, :, 0:D_HEAD],
                             vp7[tb].rearrange("p (h d) -> p h d", h=HPC))

            # Q projection (full T) and K projection (kpad), per head pair:
            # out rows = [q(2j) 64 | q(2j+1) 64] so one [128, chunk] copy
            # moves both heads at once.  Only pair 0 runs in phase 1; pairs
            # 1-3 are emitted chunk-by-chunk inside phase 2.
            def pair_chunks(j):
                return ([("q", j, c * 512, 512) for c in range(T // 512)]
                        + [("k", j, c0, w) for (c0, w) in KCH])

            def emit_chunk(spec, dve_only=False, pool=None, tag="qp"):
                kind, j, c0, w = spec
                qp = (pool or qkps).tile([128, 512], F32, tag=tag, name="qp")
                srcs = xqs if kind == "q" else xks
                wsrc = wqs[j] if kind == "q" else wks[j]
                dst = qts[j] if kind == "q" else kts[j]
                for kc in range(KC):
                    nc.tensor.matmul(
                        qp[:, 0:w],
                        wsrc[:, kc, :],
                        srcs[kc][:, c0:c0 + w],
                        start=(kc == 0), stop=(kc == KC - 1),
                    )
                if dve_only:
                    nc.vector.tensor_copy(out=dst[:, c0:c0 + w],
                                          in_=qp[:, 0:w])
                else:
                    copy_alt(dst[:, c0:c0 + w], qp[:, 0:w])

            for ci, spec in enumerate(pair_chunks(0)):
                if ci % 2 == 0:
                    emit_chunk(spec)
                else:
                    emit_chunk(spec, pool=vps, tag="qp1")

        # ---- phase 2: attention per head PAIR, q in four quarters.
        # The two heads of a pair live at SBUF partitions 0-63 / 64-127 of
        # qts/kts, so their K=64 S^T matmuls land on disjoint PE row groups
        # and execute CONCURRENTLY (measured 152ns vs 467ns per N=512 mm).
        # Both heads' scores for one (kb, quarter) go into one [128,2,512]
        # PSUM tile so a single 1024-wide exp covers them.
        num_pool = ctx.enter_context(tc.tile_pool(name="num", bufs=1))
        # stacked normalized attn^T: 2 heads per tile (cat rows)
        nums = [num_pool.tile([128, T], BF16, tag=f"nm{j}", name=f"nm{j}")
                for j in range(NPAIR)]
        NQTR = T // 512          # 4 q-quarters
        # pools that outlive phase 2 (pair-3's normalize/flush interleaves
        # with the phase-3 tail)
        avsb_pool = ctx.enter_context(tc.tile_pool(name="av_sb", bufs=4))
        r_pool = ctx.enter_context(tc.tile_pool(name="r_sb", bufs=4))
        at_pool = ctx.enter_context(tc.tile_pool(name="at_sb", bufs=2))
        tps = ctx.enter_context(tc.tile_pool(name="tps", bufs=1, space="PSUM"))

        # deferred fine-grained PE work (normalize / flush), popped a
        # little per kb step so it never lumps into an ACT bubble
        deferred = []

        def pop_deferred(n, force=False):
            for _ in range(n):
                if deferred and (force or len(deferred) > 2):
                    deferred.pop(0)()

        def norm_one(av_sb, ap_tile, r0, tb, i):
            t1 = tps.tile([128, 128], F32, tag="tp", name="t1")
            nc.tensor.transpose(
                t1[:, 0:D_HEAD + 1],
                av_sb[:, i * 128:(i + 1) * 128],
                identity[0:D_HEAD + 1, 0:D_HEAD + 1],
            )
            r_sb = r_pool.tile([128, 1], F32, tag="r", name="r_sb")
            nc.vector.reciprocal(out=r_sb, in_=t1[:, D_HEAD:D_HEAD + 1])
            nc.vector.tensor_scalar_mul(
                ap_tile[:, tb, r0:r0 + 64], t1[:, 0:D_HEAD], r_sb)

        def flush_one(ap_tile, j, tb):
            t2 = tps.tile([128, 128], BF16, tag="tp", name="t2")
            nc.tensor.transpose(t2, ap_tile[:, tb, :], identB)
            nc.vector.tensor_copy(
                out=nums[j][:, tb * 128:(tb + 1) * 128], in_=t2)

        # projection work of upcoming pairs, flattened to single-mm ops
        # popped 2 per kb step as PE filler
        chunk_ops = {}   # pair j -> list of closures

        def queue_chunk_ops(j):
            ops = []
            for spec in pair_chunks(j):
                kind, jj, c0, w = spec
                cell = {}
                for kc in range(KC):
                    def mm(kc=kc, kind=kind, jj=jj, c0=c0, w=w, cell=cell):
                        if kc == 0:
                            cell["qp"] = qkps.tile([128, 512], F32,
                                                   tag="qp", name="qp")
                        srcs = xqs if kind == "q" else xks
                        wsrc = wqs[jj] if kind == "q" else wks[jj]
                        nc.tensor.matmul(
                            cell["qp"][:, 0:w],
                            wsrc[:, kc, :],
                            srcs[kc][:, c0:c0 + w],
                            start=(kc == 0), stop=(kc == KC - 1),
                        )
                    ops.append(mm)

                def cp(kind=kind, jj=jj, c0=c0, w=w, cell=cell):
                    dst = qts[jj] if kind == "q" else kts[jj]
                    nc.vector.tensor_copy(out=dst[:, c0:c0 + w],
                                          in_=cell["qp"][:, 0:w])
                ops.append(cp)
            chunk_ops[j] = ops

        def pop_chunks(n):
            for _ in range(n):
                for j in sorted(chunk_ops):
                    if chunk_ops[j]:
                        chunk_ops[j].pop(0)()
                        break

        ap_last = [None]   # pair-3's ap tile, flushed during the tail
        with tc.tile_pool(name="p_sb", bufs=5) as p_pool, \
             tc.tile_pool(name="stps", bufs=2, space="PSUM") as stps, \
             tc.tile_pool(name="avps", bufs=1, space="PSUM") as avps:

            pair_state = {}   # pair -> {"ap":, "avp_q":}
            p_tiles = {}      # (pair, qtr, kb) -> p2

            def enter_pair(pair):
                for op in chunk_ops.pop(pair, []):   # safety: leftovers
                    op()
                if pair + 1 < NPAIR:
                    queue_chunk_ops(pair + 1)
                pair_state[pair] = {
                    "ap": at_pool.tile([128, NTB, 128], BF16,
                                       tag="ap", name="ap"),
                    "avp_q": {},
                }

            def emit_st_exp(pair, qtr, kb):
                if qtr == 0 and kb == 0:
                    enter_pair(pair)
                q0 = qtr * 512
                stp2 = stps.tile([128, 2, 512], F32, tag="stp", name="stp2")
                nc.tensor.matmul(
                    stp2[:, 0, :],
                    kts[pair][0:64, kb * 128:(kb + 1) * 128],
                    qts[pair][0:64, q0:q0 + 512],
                    start=True, stop=True,
                )
                nc.tensor.matmul(
                    stp2[:, 1, :],
                    kts[pair][64:128, kb * 128:(kb + 1) * 128],
                    qts[pair][64:128, q0:q0 + 512],
                    start=True, stop=True,
                )
                p2 = p_pool.tile([128, 2, 512], BF16, tag="p", name="p2")
                nc.scalar.activation(
                    out=p2, in_=stp2,
                    func=mybir.ActivationFunctionType.Exp,
                    bias=maskb[:, kb:kb + 1], scale=0.125,
                )
                p_tiles[(pair, qtr, kb)] = p2

            def emit_av(pair, qtr, kb):
                st = pair_state[pair]
                ap_tile = st["ap"]
                avp_q = st["avp_q"]
                h0, h1 = 2 * pair, 2 * pair + 1
                if kb == 0:
                    avp_q[qtr] = (
                        avps.tile([D_HEAD + 1, 512], F32, tag="av0",
                                  name="avp0"),
                        avps.tile([D_HEAD + 1, 512], F32, tag="av1",
                                  name="avp1"),
                    )
                avp0, avp1 = avp_q[qtr]
                p2 = p_tiles.pop((pair, qtr, kb))
                nc.tensor.matmul(
                    avp0, vaugs[kb][:, h0, :], p2[:, 0, :],
                    start=(kb == 0), stop=(kb == NKB - 1),
                )
                nc.tensor.matmul(
                    avp1, vaugs[kb][:, h1, :], p2[:, 1, :],
                    start=(kb == 0), stop=(kb == NKB - 1),
                )
                if kb == NKB - 1:
                    # drain the accumulators; queue per-token-block
                    # normalize work (tb-ordered so the tail can consume)
                    av_sbs = []
                    for r0, avp in ((0, avp0), (64, avp1)):
                        av_sb = avsb_pool.tile(
                            [D_HEAD + 1, 512], F32, tag="avsb",
                            name="av_sb")
                        nc.vector.tensor_copy(out=av_sb, in_=avp)
                        av_sbs.append((r0, av_sb))
                    for i in range(4):
                        for r0, av_sb in av_sbs:
                            deferred.append(
                                lambda av_sb=av_sb, r0=r0, i=i,
                                tb=qtr * 4 + i, ap_tile=ap_tile:
                                norm_one(av_sb, ap_tile, r0, tb, i))
                    del avp_q[qtr]
                    if qtr == NQTR - 1:
                        if pair + 1 < NPAIR:
                            for tb in range(NTB):
                                deferred.append(
                                    lambda ap_tile=ap_tile, j=pair, tb=tb:
                                    flush_one(ap_tile, j, tb))
                        else:
                            ap_last[0] = ap_tile

            # ONE continuous kb stream across all pairs and quarters: the
            # ST->exp->AV lag spans every boundary so ACT never drains
            all_steps = [(p, q, k) for p in range(NPAIR)
                         for q in range(NQTR) for k in range(NKB)]
            for i, s in enumerate(all_steps):
                emit_st_exp(*s)
                if i >= LAG:
                    emit_av(*all_steps[i - LAG])
                pop_chunks(2)
                pop_deferred(2)
            for i in range(len(all_steps) - LAG, len(all_steps)):
                emit_av(*all_steps[i])
                pop_chunks(2)
                pop_deferred(2)
            # remaining deferred = pair-3's norms (tb-ordered); they and the
            # pair-3 flush interleave with the tail below

        # ---- phase 3: tail matmul  y[tok, out] = attn_cat @ wtailT,
        # pipelined per token block with pair-3's normalize/flush
        with tc.tile_pool(name="wt", bufs=1) as wt_pool, \
             tc.tile_pool(name="y_sb", bufs=3) as y_pool, \
             tc.tile_pool(name="yps", bufs=2, space="PSUM") as yps:

            def warm_keeper3():
                dmy3 = tps.tile([128, 128], F32, tag="tp", name="dmy3")
                nc.tensor.matmul(dmy3, identity, identity, start=True,
                                 stop=True)
            wts = [wt_pool.tile([128, D_MODEL], BF16, tag=f"wt{c}", name=f"wt{c}")
                   for c in range(CAT // 128)]
            for c in range(CAT // 128):
                nc.sync.dma_start(out=wts[c], in_=wtailT[c * 128:(c + 1) * 128, :])
            pop_deferred(4, force=True)   # prime norms for tb 0-1
            for tb in range(NTB):
                flush_one(ap_last[0], NPAIR - 1, tb)
                warm_keeper3()
                yp = yps.tile([128, D_MODEL], F32, tag="yp")
                for n in range(D_MODEL // 512):
                    for c in range(CAT // 128):
                        nc.tensor.matmul(
                            yp[:, n * 512:(n + 1) * 512],
                            nums[c][:, tb * 128:(tb + 1) * 128],
                            wts[c][:, n * 512:(n + 1) * 512],
                            start=(c == 0), stop=(c == CAT // 128 - 1),
                        )
                pop_deferred(2, force=True)   # norms for tb+2
                y_sb = y_pool.tile([128, D_MODEL], F32, tag="ys")
                copy_alt(y_sb, yp)
                nc.sync.dma_start(out=y[tb * 128:(tb + 1) * 128, :], in_=y_sb)

    if split_waits:
        split_excess_waits(nc)
    return nc


_NC_CACHE = {}


def _get_nc(kpad):
    if kpad not in _NC_CACHE:
        _NC_CACHE[kpad] = build_nc(kpad)
    return _NC_CACHE[kpad]


def _plan(x, mask, w_qkv, w_tail):
    """Compute KPAD from the mask and shard full inputs into 8 core maps."""
    bf = mybir.dt.np(BF16)
    x = np.asarray(x, dtype=np.float32)
    mask = np.asarray(mask, dtype=np.int32)
    w_qkv = np.asarray(w_qkv, dtype=np.float32)
    w_tail = np.asarray(w_tail, dtype=np.float32)

    idxs = [np.flatnonzero(mask[b]) for b in range(BN)]
    nk_max = max(len(i) for i in idxs)
    kpad = max(128, -(-nk_max // 128) * 128)

    # per-batch compacted k/v-side inputs
    xTs, xkTs, maskfs = [], [], []
    for b in range(BN):
        idx = idxs[b]
        xkb = np.zeros((kpad, D_MODEL), dtype=np.float32)
        xkb[:len(idx)] = x[b][idx]
        mf = np.full(kpad, -8e9, dtype=np.float32)
        mf[:len(idx)] = 0.0
        xTs.append(np.ascontiguousarray(x[b].T).astype(bf))
        xkTs.append(np.ascontiguousarray(xkb.T).astype(bf))
        maskfs.append(mf)

    w3 = w_qkv.reshape(N_HEAD, 3, D_HEAD, D_MODEL)  # [head, qkv, d, dmodel]
    in_maps = []
    for c in range(8):
        b, hg = divmod(c, 2)
        H = range(hg * HPC, (hg + 1) * HPC)
        wq = np.concatenate([w3[h, 0] for h in H], axis=0)  # [512, 1024]
        wk = np.concatenate([w3[h, 1] for h in H], axis=0)
        wv = np.concatenate([w3[h, 2] for h in H], axis=0)
        wt = w_tail[:, hg * CAT:(hg + 1) * CAT]  # [1024, 512]
        in_maps.append({
            "ident": np.eye(128, dtype=np.float32),
            "identb": np.eye(128, dtype=bf),
            "ones8": np.ones((128, HPC), dtype=bf),
            "xT": xTs[b],
            "xkT": xkTs[b],
            "maskf": maskfs[b],
            "wqT": np.ascontiguousarray(wq.T).astype(bf),
            "wkT": np.ascontiguousarray(wk.T).astype(bf),
            "wvT": np.ascontiguousarray(wv.T).astype(bf),
            "wtailT": np.ascontiguousarray(wt.T).astype(bf),
        })
    return kpad, in_maps


def kernel(x, mask, w_qkv, w_tail, b_tail):
    kpad, in_maps = _plan(x, mask, w_qkv, w_tail)
    nc = _get_nc(kpad)
    last_err = None
    for _attempt in range(3):
        try:
            res = run_bass_kernel_spmd(nc, in_maps, list(range(8))).results
            break
        except Exception as e:  # transient device/runtime errors: retry
            last_err = e
            _time.sleep(3.0)
    else:
        raise last_err
    out = np.empty((BN, T, D_MODEL), dtype=np.float32)
    b_tail = np.asarray(b_tail, dtype=np.float32)
    for b in range(BN):
        out[b] = res[2 * b]["y"] + res[2 * b + 1]["y"] + b_tail
    return out
